# revision 21
# baseline (speedup 1.0000x reference)
"""DeepGraphInfomax loss (2-layer GCN encoder, pos+neg, DGI readout) on 8 trn2 cores.

Strategy (dst-sharded pull-mode GNN aggregation):
  - Nodes (dst rows) sharded contiguously across 8 cores (12500 each).
  - pos/neg feature streams fused into 128-wide rows: X2[r] = [x[r] | x[perm[r]]].
  - W1/W2 moved *after* aggregation (A @ (X W) == (A @ X) W), so layer-1 gathers
    read the bf16 input X2 directly and layer-2 gathers read R2 = relu(out1).
  - Per core, edges are sorted by (src-chunk, dst, src) and padded into uniform
    1024-slot groups; each group of 8x128-slot tiles is segment-reduced with a
    one-hot matmul: psum[128f x 128d] += Hg[128s x 128f]^T @ P[128s x 128d],
    P = (iota == dstl) * norm, then added into a feature-major f32 SBUF
    accumulator at a dynamic dst-window offset.
  - dma_gather (SWDGE custom op) does the 256B-row gathers; int16 indices are
    made valid by splitting the source rows into 4 chunks of 2*LDIM (< 32768).
  - Layer-2 sources are exchanged with one AllGather of relu(out1) (bf16).
  - DGI readout (summary / W_dgi / softplus losses) computed on device with two
    tiny AllReduces ([128,1] column sums and the final scalar).

Host-side preprocessing only manipulates integer graph structure (sorting,
degree counts, packing, index wrapping) and stages dtype-cast copies of the
inputs; all floating-point math of the reference runs on device.
"""

import sys

for _p in ("/opt/trn_rl_repo", "/root/.axon_site/_ro/trn_rl_repo"):
    if _p not in sys.path:
        sys.path.insert(0, _p)

from contextlib import ExitStack

import ml_dtypes
import numpy as np

import concourse.bass as bass
import concourse.bacc as bacc
import concourse.mybir as mybir
import concourse.tile as tile
from concourse.bass_utils import run_bass_kernel_spmd

BF16 = ml_dtypes.bfloat16
F32 = np.float32

C = 8            # cores
D = 64           # hidden dim
DF = 2 * D       # fused pos|neg width
SLOTS_PER_TILE = 128
TILES_PER_GROUP = 4
GROUPS_PER_CALL = 8
SLOTS_PER_GROUP = SLOTS_PER_TILE * TILES_PER_GROUP      # 1024
SLOTS_PER_CALL = SLOTS_PER_GROUP * GROUPS_PER_CALL      # 4096
TILES_PER_CALL = TILES_PER_GROUP * GROUPS_PER_CALL      # 32
NSEC = 4
SCB = 7            # gather calls per scatter batch
PAD_DEG = 1e30   # pad-slot degree product -> norm ~ 1e-15 ~ 0


class Geo:
    """Problem geometry. nt dst-tiles of 128 per core; source row spaces are
    padded to xrows = 8*ldim = 4*sec so int16 chunk-local gather indices fit."""

    def __init__(self, npc, nreal):
        self.npc = npc                      # real nodes per core
        self.nreal = nreal                  # total real nodes (= 8*npc)
        self.nt = -(-npc // 128)            # dst tiles per core
        self.ldim = 128 * self.nt           # padded dsts per core
        self.xrows = 8 * self.ldim          # padded source-row space
        self.sec = self.xrows // NSEC       # chunk size (must be < 32768)
        self.accw = self.ldim + 128         # acc free width (window overhang)
        assert self.sec < 32768


def _group_walk(d):
    """Split a dst-sorted run into groups of <=1024 slots whose dst window
    stays < 128 wide and that never split one dst across groups (so groups
    within a section have disjoint dsts). Returns list of (start, end)."""
    out = []
    i = 0
    n = len(d)
    while i < n:
        j = min(i + SLOTS_PER_GROUP, n)
        lim = np.searchsorted(d, d[i] + 128, side="left")
        j = min(j, lim)
        if j < n:
            j = int(np.searchsorted(d, d[j - 1], side="left")) \
                if d[j] == d[j - 1] else j
            if j <= i:  # single dst larger than a group (impossible-ish)
                j = min(i + SLOTS_PER_GROUP, n)
        out.append((i, int(j)))
        i = int(j)
    return out


def _preprocess(g, x, W1, b1, W2, b2, W_dgi, edge_index, perm):
    """Build per-core device inputs. Integer index work + dtype staging only."""
    row = np.asarray(edge_index[0], dtype=np.int64)
    col = np.asarray(edge_index[1], dtype=np.int64)
    perm = np.asarray(perm, dtype=np.int64)
    N = g.nreal

    deg = np.bincount(col, minlength=N).astype(np.int64) + 1  # ref: in-deg + 1
    # fold self-loops in as explicit edges (degp = deg^2 -> weight 1/deg)
    gids = np.arange(N, dtype=np.int64)
    row = np.concatenate([row, gids])
    col = np.concatenate([col, gids])

    # fused bf16 feature rows, padded to xrows
    X2 = np.zeros((g.xrows, DF), dtype=BF16)
    X2[:N, :D] = x.astype(BF16)
    X2[:N, D:] = x[perm].astype(BF16)

    core_of = col // g.npc
    r2 = (row // g.npc) * g.ldim + (row % g.npc)  # row id in R2-space

    # pass 1: per (core, layer, section) sorted groups
    per_core = []
    for k in range(C):
        m = core_of == k
        rk = row[m]
        dk = col[m] - k * g.npc
        r2k = r2[m]
        layers = []
        for srcid in (rk, r2k):
            sec = srcid // g.sec
            order = np.lexsort((srcid, dk, sec))
            ss, ds_, rs = srcid[order], dk[order], rk[order]
            sec_s = sec[order]
            sections = []
            for s in range(NSEC):
                lo = np.searchsorted(sec_s, s, side="left")
                hi = np.searchsorted(sec_s, s, side="right")
                dloc = ds_[lo:hi]
                groups = _group_walk(dloc)
                sections.append((lo, hi, groups))
            layers.append((ss, ds_, rs, sections))
        per_core.append(layers)

    # uniform calls per (layer, section) across cores
    calls = []
    for li in range(2):
        cl = []
        for s in range(NSEC):
            mx = 0
            for k in range(C):
                ngr = len(per_core[k][li][3][s][2])
                mx = max(mx, -(-ngr // GROUPS_PER_CALL))
            cl.append(mx)
        calls.append(tuple(cl))

    # pass 2: build padded per-core arrays
    deg_f = deg.astype(np.float64)
    ins = []
    for k in range(C):
        d_in = {}
        for li in range(2):
            ss, ds_, rs, sections = per_core[k][li]
            ctot = sum(calls[li])
            S = ctot * SLOTS_PER_CALL
            G = ctot * GROUPS_PER_CALL
            idx = np.zeros(S, dtype=np.int16)
            dstl = np.zeros(S, dtype=np.int32)
            degp = np.full(S, PAD_DEG, dtype=F32)
            # scatter row targets: group result row j -> base+j (real dsts)
            # or a dump row (>= ldim) for unused rows; dump collisions only
            # ever receive zero-adds or races on garbage.
            scidx = np.empty(G * 128, dtype=np.int16)
            jj = np.arange(128)
            goff = 0
            for s in range(NSEC):
                lo, hi, groups = sections[s]
                for gi in range(calls[li][s] * GROUPS_PER_CALL):
                    grow = (goff + gi) * 128
                    if gi < len(groups):
                        a, b = groups[gi]
                        gslot = (goff + gi) * SLOTS_PER_GROUP
                        n = b - a
                        base = int(ds_[lo + a])
                        ndst = int(ds_[lo + b - 1]) - base + 1
                        sl = slice(gslot, gslot + n)
                        idx[sl] = (ss[lo + a : lo + b] - s * g.sec).astype(
                            np.int16
                        )
                        dstl[sl] = ds_[lo + a : lo + b] - base
                        degp[sl] = (
                            deg_f[rs[lo + a : lo + b]]
                            * deg_f[k * g.npc + ds_[lo + a : lo + b]]
                        ).astype(F32)
                        rows = (base + jj).copy()
                        rows[ndst:] = g.ldim + jj[ndst:]
                        scidx[grow : grow + 128] = rows.astype(np.int16)
                    else:
                        scidx[grow : grow + 128] = (g.ldim + jj).astype(
                            np.int16
                        )
                goff += calls[li][s] * GROUPS_PER_CALL
            assert dstl.min() >= 0 and dstl.max() < 128
            # wrapped int16 index layout: slot j -> [j%16, j//16], replicated x8
            idx_w = np.tile(idx.reshape(-1, 16).T, (8, 1)).astype(np.int16)
            sc_w = np.tile(scidx.reshape(-1, 16).T, (8, 1)).astype(np.int16)
            dstl_w = np.ascontiguousarray(
                dstl.reshape(-1, 128).T.astype(F32)
            )
            degp_w = np.ascontiguousarray(degp.reshape(-1, 128).T)
            L = li + 1
            d_in[f"idx{L}"] = np.ascontiguousarray(idx_w)
            d_in[f"scidx{L}"] = np.ascontiguousarray(sc_w)
            d_in[f"dstl{L}"] = dstl_w
            d_in[f"degp{L}"] = degp_w

        # valid-dst mask [128, nt]
        mk = (np.arange(g.ldim) < g.npc).astype(F32)
        d_in["mask"] = np.ascontiguousarray(mk.reshape(g.nt, 128).T)

        ins.append(d_in)

    # shared constants
    iota = np.tile(np.arange(128, dtype=F32), (128, 1)).astype(BF16)
    ident = np.eye(128, dtype=BF16)
    wc1 = np.zeros((DF, DF), dtype=F32)
    wc1[:D, :D] = W1
    wc1[D:, D:] = W1
    wc2 = np.zeros((DF, DF), dtype=F32)
    wc2[:D, :D] = W2
    wc2[D:, D:] = W2
    bc1 = np.concatenate([b1, b1]).astype(F32).reshape(DF, 1)
    bc2 = np.concatenate([b2, b2]).astype(F32).reshape(DF, 1)
    # ws2 = (wdgi_stack^T @ summary) * colmask ; wdgi_stack[k, p] = W_dgi[p%64, k]
    wstack = np.zeros((D, DF), dtype=F32)
    wstack[:, :D] = W_dgi.T
    wstack[:, D:] = W_dgi.T
    colmask = np.zeros((DF, 2), dtype=F32)
    colmask[:D, 0] = 1.0
    colmask[D:, 1] = 1.0
    # last-tile pad zeroing pattern, [128,128] along free dim
    nvalid_last = g.npc - (g.nt - 1) * 128
    lastmask = np.tile(
        (np.arange(128) < nvalid_last).astype(F32), (128, 1)
    )
    shared = {
        "x2": X2,
        "iota": iota,
        "ident": ident,
        "identf": np.eye(128, dtype=F32),
        "wc1": wc1,
        "wc2": wc2,
        "bc1": bc1,
        "bc2": bc2,
        "wstack": wstack,
        "colmask": colmask,
        "lastmask": lastmask,
        "ones": np.ones((128, 1), dtype=F32),
    }
    for d_in in ins:
        d_in.update(shared)
    return ins, calls


def _build(g, calls1, calls2):
    dt = mybir.dt
    nc = bacc.Bacc(
        "TRN2", target_bir_lowering=False, debug=False, num_devices=C
    )

    def din(name, shape, dty):
        return nc.dram_tensor(name, list(shape), dty, kind="ExternalInput").ap()

    ct1 = sum(calls1)
    ct2 = sum(calls2)
    x2 = din("x2", (g.xrows, DF), dt.bfloat16)
    idx_d = [
        din("idx1", (128, ct1 * 256), dt.int16),
        din("idx2", (128, ct2 * 256), dt.int16),
    ]
    dstl_d = [
        din("dstl1", (128, ct1 * TILES_PER_CALL), dt.float32),
        din("dstl2", (128, ct2 * TILES_PER_CALL), dt.float32),
    ]
    degp_d = [
        din("degp1", (128, ct1 * TILES_PER_CALL), dt.float32),
        din("degp2", (128, ct2 * TILES_PER_CALL), dt.float32),
    ]
    scidx_d = [
        din("scidx1", (128, ct1 * GROUPS_PER_CALL * 8), dt.int16),
        din("scidx2", (128, ct2 * GROUPS_PER_CALL * 8), dt.int16),
    ]
    identf_d = din("identf", (128, 128), dt.float32)
    mask_d = din("mask", (128, g.nt), dt.float32)
    iota_d = din("iota", (128, 128), dt.bfloat16)
    ident_d = din("ident", (128, 128), dt.bfloat16)
    wc_d = [din("wc1", (DF, DF), dt.float32), din("wc2", (DF, DF), dt.float32)]
    bc_d = [din("bc1", (DF, 1), dt.float32), din("bc2", (DF, 1), dt.float32)]
    wstack_d = din("wstack", (D, DF), dt.float32)
    colmask_d = din("colmask", (DF, 2), dt.float32)
    lastmask_d = din("lastmask", (128, 128), dt.float32)
    ones_d = din("ones", (128, 1), dt.float32)
    loss_out = nc.dram_tensor("loss", [1, 16], dt.float32, kind="ExternalOutput").ap()

    inv_n = 1.0 / float(g.nreal)
    rg = [list(range(C))]

    with tile.TileContext(nc) as tc, ExitStack() as ctx:
        dram = ctx.enter_context(tc.tile_pool(name="dram", bufs=1, space="DRAM"))
        r2shard = dram.tile([g.ldim, DF], dt.bfloat16, tag="r2shard")
        r2full = dram.tile([g.xrows, DF], dt.bfloat16, tag="r2full", addr_space="Shared")
        cs_in = dram.tile([128, 1], dt.float32, tag="cs_in")
        cs_out = dram.tile([128, 1], dt.float32, tag="cs_out", addr_space="Shared")
        ls_in = dram.tile([1, 16], dt.float32, tag="ls_in")
        ls_out = dram.tile([1, 16], dt.float32, tag="ls_out", addr_space="Shared")

        const = ctx.enter_context(tc.tile_pool(name="const", bufs=1))

        def cload(ap_dram, shape, dty, tag):
            t = const.tile(list(shape), dty, tag=tag)
            nc.sync.dma_start(t[:], ap_dram)
            return t

        iota_sb = cload(iota_d, (128, 128), dt.bfloat16, "iota")
        ident_sb = cload(ident_d, (128, 128), dt.bfloat16, "ident")
        identf_sb = cload(identf_d, (128, 128), dt.float32, "identf")
        wc_sb = [
            cload(wc_d[0], (DF, DF), dt.float32, "wc1"),
            cload(wc_d[1], (DF, DF), dt.float32, "wc2"),
        ]
        bc_sb = [
            cload(bc_d[0], (DF, 1), dt.float32, "bc1"),
            cload(bc_d[1], (DF, 1), dt.float32, "bc2"),
        ]
        wstack_sb = cload(wstack_d, (D, DF), dt.float32, "wstack")
        colmask_sb = cload(colmask_d, (DF, 2), dt.float32, "colmask")
        lastmask_sb = cload(lastmask_d, (128, 128), dt.float32, "lastmask")
        ones_sb = cload(ones_d, (128, 1), dt.float32, "ones")
        mask_sb = cload(mask_d, (128, g.nt), dt.float32, "mask")

        big = ctx.enter_context(tc.tile_pool(name="big", bufs=1))
        z_sb = big.tile([128, g.ldim], dt.float32, tag="z_sb")
        # DRAM accumulator, d-major: rows [0, ldim) real dsts, rows
        # [ldim, ldim+128) dump for unused scatter rows.
        accr = g.ldim + 128
        zrows = 128 * max(1, min(10, accr // 128))
        while accr % zrows != 0:
            zrows -= 128
        acc_dram = dram.tile([accr, DF], dt.float32, tag="acc_dram")
        zt = big.tile([128, (zrows // 128) * DF], dt.float32, tag="zt")
        nc.vector.memset(zt[:], 0.0)

        # slot-norm metadata pool; tags shared between layers (sequential use)
        meta = ctx.enter_context(tc.tile_pool(name="meta", bufs=1))

        stg = ctx.enter_context(tc.tile_pool(name="stg", bufs=2))
        idxp = ctx.enter_context(tc.tile_pool(name="idxp", bufs=3))
        gpool = ctx.enter_context(tc.tile_pool(name="gpool", bufs=3))
        ppool = ctx.enter_context(tc.tile_pool(name="ppool", bufs=6))
        psg = ctx.enter_context(tc.tile_pool(name="psg", bufs=3, space="PSUM"))
        pst = ctx.enter_context(tc.tile_pool(name="pst", bufs=2, space="PSUM"))
        psm = ctx.enter_context(tc.tile_pool(name="psm", bufs=2, space="PSUM"))
        psl = ctx.enter_context(tc.tile_pool(name="psl", bufs=1, space="PSUM"))
        work = ctx.enter_context(tc.tile_pool(name="work", bufs=3))
        outp = ctx.enter_context(tc.tile_pool(name="outp", bufs=3))

        def aggregate(li, calls, src_dram):
            ct = sum(calls)
            dg = meta.tile([128, ct * TILES_PER_CALL], dt.float32, tag="dg")
            nc.sync.dma_start(dg[:], degp_d[li])
            nc.vector.reciprocal(dg[:], dg[:])
            wv = meta.tile([128, ct * TILES_PER_CALL], dt.float32, tag="wv")
            nc.scalar.sqrt(wv[:], dg[:])
            dl = meta.tile([128, ct * TILES_PER_CALL], dt.float32, tag="dl")
            nc.sync.dma_start(dl[:], dstl_d[li])
            # zero the accumulator
            nz = accr // zrows
            for zi in range(nz):
                nc.sync.dma_start(
                    acc_dram[zi * zrows : (zi + 1) * zrows, :].rearrange(
                        "(p a) f -> p (a f)", p=128
                    ),
                    zt[:],
                )
            cglob = 0
            for s in range(NSEC):
                src_sec = src_dram[s * g.sec : (s + 1) * g.sec, :]
                # scatter batches of <= SCB calls; groups within one section
                # have disjoint dsts, so batch scatters of the same section
                # cannot race each other.
                nbat = -(-calls[s] // SCB)
                batches = []
                c0 = 0
                for _b in range(nbat):
                    c1 = min(c0 + SCB, calls[s])
                    batches.append((c0, c1))
                    c0 = c1
                for (c0, c1) in batches:
                  nsg = (c1 - c0) * GROUPS_PER_CALL
                  stage = stg.tile(
                      [128, SCB * GROUPS_PER_CALL, DF], dt.float32, tag="stage"
                  )
                  sci = idxp.tile(
                      [128, SCB * GROUPS_PER_CALL * 8], dt.int16, tag="sci"
                  )
                  nc.sync.dma_start(
                      sci[:, : nsg * 8],
                      scidx_d[li][
                          :, cglob * GROUPS_PER_CALL * 8 :
                          (cglob * GROUPS_PER_CALL + nsg) * 8
                      ],
                  )
                  g0 = cglob * GROUPS_PER_CALL
                  for _c in range(c1 - c0):
                      it = idxp.tile([128, 256], dt.int16, tag="it")
                      nc.sync.dma_start(
                          it[:], idx_d[li][:, cglob * 256 : (cglob + 1) * 256]
                      )
                      gt = gpool.tile(
                          [128, TILES_PER_CALL, DF], dt.bfloat16, tag="gt"
                      )
                      nc.gpsimd.dma_gather(
                          gt[:], src_sec, it[:], SLOTS_PER_CALL, SLOTS_PER_CALL,
                          DF, single_packet=False,
                      )
                      for q in range(GROUPS_PER_CALL):
                          ps = psg.tile([128, 128], dt.float32, tag="ps")
                          for t in range(TILES_PER_GROUP):
                              tc_i = q * TILES_PER_GROUP + t
                              col = cglob * TILES_PER_CALL + tc_i
                              P = ppool.tile([128, 128], dt.bfloat16, tag="P")
                              nc.vector.tensor_scalar(
                                  P[:],
                                  iota_sb[:],
                                  dl[:, col : col + 1],
                                  wv[:, col : col + 1],
                                  mybir.AluOpType.is_equal,
                                  mybir.AluOpType.mult,
                              )
                              nc.tensor.matmul(
                                  ps[:],
                                  lhsT=P[:],
                                  rhs=gt[:, tc_i, :],
                                  start=(t == 0),
                                  stop=(t == TILES_PER_GROUP - 1),
                              )
                          gl = cglob * GROUPS_PER_CALL + q - g0
                          nc.vector.tensor_copy(stage[:, gl, :], ps[:])
                      cglob += 1
                  nc.gpsimd.dma_scatter_add(
                      acc_dram[:],
                      stage[:, :nsg, :],
                      sci[:, : nsg * 8],
                      nsg * 128,
                      nsg * 128,
                      DF,
                      single_packet=False,
                  )

        def post(li, to_r2):
            # u = acc (self-loop folded into edges); out = wc^T @ u^T; +b
            for dti in range(g.nt):
                sl = slice(dti * 128, (dti + 1) * 128)
                at = work.tile([128, DF], dt.float32, tag="at")
                nc.sync.dma_start(at[:], acc_dram[sl, :])
                tp = pst.tile([128, 128], dt.float32, tag="tp")
                nc.tensor.transpose(tp[:], at[:], identf_sb[:])
                ut = work.tile([128, 128], dt.float32, tag="ut")
                nc.vector.tensor_copy(ut[:], tp[:])
                po = psm.tile([128, 128], dt.float32, tag="po")
                nc.tensor.matmul(
                    po[:], lhsT=wc_sb[li][:], rhs=ut[:], start=True, stop=True
                )
                if to_r2:
                    rb = outp.tile([128, 128], dt.bfloat16, tag="rb")
                    nc.vector.tensor_scalar(
                        rb[:],
                        po[:],
                        bc_sb[li][:],
                        0.0,
                        mybir.AluOpType.add,
                        mybir.AluOpType.max,
                    )
                    rt = outp.tile([128, 128], dt.bfloat16, tag="rt")
                    nc.sync.dma_start_transpose(rt[:], rb[:])
                    nc.sync.dma_start(r2shard[sl, :], rt[:])
                else:
                    nc.vector.tensor_scalar(
                        z_sb[:, sl],
                        po[:],
                        bc_sb[li][:],
                        None,
                        mybir.AluOpType.add,
                    )
                    if dti == g.nt - 1:
                        nc.vector.tensor_tensor(
                            z_sb[:, sl],
                            z_sb[:, sl],
                            lastmask_sb[:],
                            op=mybir.AluOpType.mult,
                        )

        aggregate(0, calls1, x2)
        post(0, to_r2=True)

        nc.gpsimd.collective_compute(
            "AllGather",
            mybir.AluOpType.bypass,
            replica_groups=rg,
            ins=[r2shard[:].opt()],
            outs=[r2full[:].opt()],
        )

        aggregate(1, calls2, r2full[:])
        post(1, to_r2=False)

        # ---- DGI readout ----
        fin = ctx.enter_context(tc.tile_pool(name="fin", bufs=1))
        cs = fin.tile([128, 1], dt.float32, tag="cs")
        nc.vector.reduce_sum(cs[:], z_sb[:], axis=mybir.AxisListType.X)
        nc.sync.dma_start(cs_in[:], cs[:])
        nc.gpsimd.collective_compute(
            "AllReduce",
            mybir.AluOpType.add,
            replica_groups=rg,
            ins=[cs_in[:].opt()],
            outs=[cs_out[:].opt()],
        )
        cst = fin.tile([128, 1], dt.float32, tag="cst")
        nc.sync.dma_start(cst[:], cs_out[:])
        summ = fin.tile([128, 1], dt.float32, tag="summ")
        nc.scalar.activation(
            summ[:], cst[:], mybir.ActivationFunctionType.Sigmoid, scale=inv_n
        )
        wsps = psl.tile([DF, 1], dt.float32, tag="pls")
        nc.tensor.matmul(
            wsps[:], lhsT=wstack_sb[:], rhs=summ[0:D, 0:1], start=True, stop=True
        )
        ws2 = fin.tile([DF, 2], dt.float32, tag="ws2")
        nc.vector.tensor_tensor(
            ws2[:],
            colmask_sb[:],
            wsps[:].to_broadcast([DF, 2]),
            op=mybir.AluOpType.mult,
        )
        tp_sb = fin.tile([128, g.nt], dt.float32, tag="tp_sb")
        tn_sb = fin.tile([128, g.nt], dt.float32, tag="tn_sb")
        for dti in range(g.nt):
            sl = slice(dti * 128, (dti + 1) * 128)
            tps = psl.tile([128, 2], dt.float32, tag="pls")
            nc.tensor.matmul(
                tps[:], lhsT=z_sb[:, sl], rhs=ws2[:], start=True, stop=True
            )
            nc.vector.tensor_copy(tp_sb[:, dti : dti + 1], tps[:, 0:1])
            nc.vector.tensor_copy(tn_sb[:, dti : dti + 1], tps[:, 1:2])

        # softplus(sgn*t) = relu(sgn*t) + ln1p(exp(-|t|)); deg-7 poly for ln1p
        LN1P = [
            5.62195900721818e-07, 0.9999574870750696, -0.4992065685478763,
            0.32697310001391783, -0.2228362583278401, 0.13076503250360005,
            -0.05262485136716543, 0.010119082927575069,
        ]

        def softplus_of(t_in, sgn, tagp):
            neg = fin.tile([128, g.nt], dt.float32, tag=f"{tagp}neg")
            nc.vector.tensor_scalar(
                neg[:], t_in[:], -1.0, None, mybir.AluOpType.mult
            )
            ab = fin.tile([128, g.nt], dt.float32, tag=f"{tagp}ab")
            nc.vector.tensor_tensor(ab[:], t_in[:], neg[:], op=mybir.AluOpType.max)
            uu = fin.tile([128, g.nt], dt.float32, tag=f"{tagp}uu")
            nc.scalar.activation(
                uu[:], ab[:], mybir.ActivationFunctionType.Exp, scale=-1.0
            )
            pp_ = fin.tile([128, g.nt], dt.float32, tag=f"{tagp}pp")
            nc.vector.tensor_scalar(
                pp_[:], uu[:], LN1P[7], LN1P[6],
                mybir.AluOpType.mult, mybir.AluOpType.add,
            )
            pm = fin.tile([128, g.nt], dt.float32, tag=f"{tagp}pm")
            for ci in range(5, -1, -1):
                nc.vector.tensor_tensor(
                    pm[:], pp_[:], uu[:], op=mybir.AluOpType.mult
                )
                nc.vector.tensor_scalar(
                    pp_[:], pm[:], LN1P[ci], None, mybir.AluOpType.add
                )
            rl = fin.tile([128, g.nt], dt.float32, tag=f"{tagp}rl")
            nc.vector.tensor_scalar(
                rl[:], (t_in if sgn > 0 else neg)[:], 0.0, None,
                mybir.AluOpType.max,
            )
            res = fin.tile([128, g.nt], dt.float32, tag=f"{tagp}res")
            nc.vector.tensor_tensor(res[:], rl[:], pp_[:], op=mybir.AluOpType.add)
            return res

        spp = softplus_of(tp_sb, -1, "sp")   # softplus(-t_pos)
        spn = softplus_of(tn_sb, +1, "sn")   # softplus(t_neg)
        ssum = fin.tile([128, g.nt], dt.float32, tag="ssum")
        nc.vector.tensor_tensor(ssum[:], spp[:], spn[:], op=mybir.AluOpType.add)
        nc.vector.tensor_tensor(
            ssum[:], ssum[:], mask_sb[:], op=mybir.AluOpType.mult
        )
        srow = fin.tile([128, 1], dt.float32, tag="srow")
        nc.vector.reduce_sum(srow[:], ssum[:], axis=mybir.AxisListType.X)
        tot = psl.tile([1, 1], dt.float32, tag="pls")
        nc.tensor.matmul(
            tot[:], lhsT=srow[:], rhs=ones_sb[:], start=True, stop=True
        )
        lsb = fin.tile([1, 16], dt.float32, tag="lsb")
        nc.vector.memset(lsb[:], 0.0)
        nc.vector.tensor_copy(lsb[0:1, 0:1], tot[:])
        nc.sync.dma_start(ls_in[:], lsb[:])
        nc.gpsimd.collective_compute(
            "AllReduce",
            mybir.AluOpType.add,
            replica_groups=rg,
            ins=[ls_in[:].opt()],
            outs=[ls_out[:].opt()],
        )
        lsf = fin.tile([1, 16], dt.float32, tag="lsf")
        nc.sync.dma_start(lsf[:], ls_out[:])
        lout = fin.tile([1, 16], dt.float32, tag="lout")
        nc.scalar.activation(
            lout[:], lsf[:], mybir.ActivationFunctionType.Copy, scale=inv_n
        )
        nc.sync.dma_start(loss_out, lout[:])

    nc.compile()
    return nc


_prog_cache = {}


def _get_prog(g, calls1, calls2):
    key = (g.npc, g.nreal, calls1, calls2)
    if key not in _prog_cache:
        _prog_cache[key] = _build(g, calls1, calls2)
    return _prog_cache[key]


def run(inputs, npc, nreal, trace=False):
    g = Geo(npc, nreal)
    in_maps, calls = _preprocess(g, **inputs)
    nc = _get_prog(g, calls[0], calls[1])
    res = run_bass_kernel_spmd(
        nc, in_maps, core_ids=list(range(C)), trace=trace
    )
    loss = res.results[0]["loss"][0, 0]
    return np.float32(loss), res


def kernel(**inputs):
    out, _ = run(inputs, npc=12500, nreal=100000)
    return out


def _make_sharded_exec(nc, in_maps, reps=1):
    """Reusable jitted shard_map executor mirroring bass2jax's multi-core
    path, with device-resident inputs. With reps>1 the NEFF is executed
    reps times inside one dispatch so per-execution time can be resolved
    above the ~200ms axon dispatch floor."""
    import jax
    from jax.experimental.shard_map import shard_map
    from jax.sharding import Mesh, NamedSharding, PartitionSpec

    from concourse import bass2jax, mybir as _mb

    bass2jax.install_neuronx_cc_hook()
    partition_name = (
        nc.partition_id_tensor.name if nc.partition_id_tensor else None
    )
    in_names, out_names, out_avals, zero_shapes = [], [], [], []
    for alloc in nc.m.functions[0].allocations:
        if not isinstance(alloc, _mb.MemoryLocationSet):
            continue
        name = alloc.memorylocations[0].name
        if alloc.kind == "ExternalInput":
            if name != partition_name:
                in_names.append(name)
        elif alloc.kind == "ExternalOutput":
            shape = tuple(alloc.tensor_shape)
            dty = _mb.dt.np(alloc.dtype)
            out_names.append(name)
            out_avals.append(jax.core.ShapedArray(shape, dty))
            zero_shapes.append((shape, dty))
    n_params = len(in_names)
    n_outs = len(out_avals)
    all_names = list(in_names) + list(out_names)
    if partition_name is not None:
        all_names.append(partition_name)
    donate = tuple(range(n_params, n_params + n_outs * reps))

    assert reps == 1  # the neuronx_cc hook allows one bass_exec per module

    def _body(*args):
        operands = list(args)
        if partition_name is not None:
            operands.append(bass2jax.partition_id_tensor())
        outs = bass2jax._bass_exec_p.bind(
            *operands,
            out_avals=tuple(out_avals),
            in_names=tuple(all_names),
            out_names=tuple(out_names),
            lowering_input_output_aliases=(),
            sim_require_finite=True,
            sim_require_nnan=True,
            nc=nc,
        )
        return tuple(outs)

    devices = jax.devices()[:C]
    mesh = Mesh(np.array(devices), ("core",))
    spec = PartitionSpec("core")
    sharded = jax.jit(
        shard_map(
            _body,
            mesh=mesh,
            in_specs=(spec,) * (n_params + n_outs * reps),
            out_specs=(spec,) * n_outs,
            check_rep=False,
        ),
        donate_argnums=donate,
        keep_unused=True,
    )
    shard = NamedSharding(mesh, spec)
    concat_in = [
        jax.device_put(
            np.concatenate([np.asarray(m[nm]) for m in in_maps], axis=0), shard
        )
        for nm in in_names
    ]

    def launch():
        zeros = [
            jax.device_put(np.zeros((C * s[0], *s[1:]), d), shard)
            for (s, d) in zero_shapes
        ]
        return sharded(*concat_in, *zeros)

    def fetch(outs):
        jax.block_until_ready(outs)
        return {
            nm: np.asarray(outs[i]).reshape(C, *out_avals[i].shape)[0]
            for i, nm in enumerate(out_names)
        }

    def run_once():
        return fetch(launch())

    run_once.launch = launch
    run_once.fetch = fetch
    return run_once


def bench(inputs, npc=12500, nreal=100000, iters=6):
    import time

    g = Geo(npc, nreal)
    t0 = time.time()
    in_maps, calls = _preprocess(g, **inputs)
    t1 = time.time()
    nc = _get_prog(g, calls[0], calls[1])
    t2 = time.time()
    run_1 = _make_sharded_exec(nc, in_maps)
    out = run_1()  # warmup: compiles + loads NEFF
    t3 = time.time()
    t1s = []
    for _ in range(iters):
        ta = time.time()
        out = run_1()
        t1s.append(time.time() - ta)
    # pipelined async launches: marginal cost per launch approximates
    # NEFF execution + per-exec overhead without the full dispatch floor
    K = 48
    ta = time.time()
    pend = [run_1.launch() for _ in range(K)]
    import jax as _jax
    _jax.block_until_ready(pend)
    tK = time.time() - ta
    per = (tK - min(t1s)) / (K - 1)
    print(
        f"preprocess {t1-t0:.1f}s  build {t2-t1:.1f}s  warmup {t3-t2:.1f}s\n"
        f"  1-shot ms: {[round(t*1e3,2) for t in t1s]}\n"
        f"  {K} pipelined: total {tK*1e3:.1f} ms -> marginal {per*1e3:.3f} ms"
    )
    return np.float32(out["loss"][0, 0]), per



# revision 22
# speedup vs baseline: 1.5574x; 1.5574x over previous
"""DeepGraphInfomax loss (2-layer GCN encoder, pos+neg, DGI readout) on 8 trn2 cores.

Strategy (dst-sharded pull-mode GNN aggregation):
  - Nodes (dst rows) sharded contiguously across 8 cores (12500 each).
  - pos/neg feature streams fused into 128-wide rows: X2[r] = [x[r] | x[perm[r]]].
  - W1/W2 moved *after* aggregation (A @ (X W) == (A @ X) W), so layer-1 gathers
    read the bf16 input X2 directly and layer-2 gathers read R2 = relu(out1).
  - Per core, edges are sorted by (src-chunk, dst, src) and padded into uniform
    1024-slot groups; each group of 8x128-slot tiles is segment-reduced with a
    one-hot matmul: psum[128f x 128d] += Hg[128s x 128f]^T @ P[128s x 128d],
    P = (iota == dstl) * norm, then added into a feature-major f32 SBUF
    accumulator at a dynamic dst-window offset.
  - dma_gather (SWDGE custom op) does the 256B-row gathers; int16 indices are
    made valid by splitting the source rows into 4 chunks of 2*LDIM (< 32768).
  - Layer-2 sources are exchanged with one AllGather of relu(out1) (bf16).
  - DGI readout (summary / W_dgi / softplus losses) computed on device with two
    tiny AllReduces ([128,1] column sums and the final scalar).

Host-side preprocessing only manipulates integer graph structure (sorting,
degree counts, packing, index wrapping) and stages dtype-cast copies of the
inputs; all floating-point math of the reference runs on device.
"""

import sys

for _p in ("/opt/trn_rl_repo", "/root/.axon_site/_ro/trn_rl_repo"):
    if _p not in sys.path:
        sys.path.insert(0, _p)

from contextlib import ExitStack

import ml_dtypes
import numpy as np

import concourse.bass as bass
import concourse.bacc as bacc
import concourse.mybir as mybir
import concourse.tile as tile
from concourse.bass_utils import run_bass_kernel_spmd

BF16 = ml_dtypes.bfloat16
F32 = np.float32

C = 8            # cores
D = 64           # hidden dim
DF = 2 * D       # fused pos|neg width
SLOTS_PER_TILE = 128
TILES_PER_GROUP = 4
GROUPS_PER_CALL = 8
SLOTS_PER_GROUP = SLOTS_PER_TILE * TILES_PER_GROUP      # 1024
SLOTS_PER_CALL = SLOTS_PER_GROUP * GROUPS_PER_CALL      # 4096
TILES_PER_CALL = TILES_PER_GROUP * GROUPS_PER_CALL      # 32
NSEC = 4
SCB = 7            # gather calls per scatter batch
PAD_DEG = 1e30   # pad-slot degree product -> norm ~ 1e-15 ~ 0


class Geo:
    """Problem geometry. nt dst-tiles of 128 per core; source row spaces are
    padded to xrows = 8*ldim = 4*sec so int16 chunk-local gather indices fit."""

    def __init__(self, npc, nreal):
        self.npc = npc                      # real nodes per core
        self.nreal = nreal                  # total real nodes (= 8*npc)
        self.nt = -(-npc // 128)            # dst tiles per core
        self.ldim = 128 * self.nt           # padded dsts per core
        self.xrows = 8 * self.ldim          # padded source-row space
        self.sec = self.xrows // NSEC       # chunk size (must be < 32768)
        self.accw = self.ldim + 128         # acc free width (window overhang)
        assert self.sec < 32768


def _group_walk(d):
    """Split a dst-sorted run into groups of <=1024 slots whose dst window
    stays < 128 wide and that never split one dst across groups (so groups
    within a section have disjoint dsts). Returns list of (start, end)."""
    out = []
    i = 0
    n = len(d)
    while i < n:
        j = min(i + SLOTS_PER_GROUP, n)
        lim = np.searchsorted(d, d[i] + 128, side="left")
        j = min(j, lim)
        if j < n:
            j = int(np.searchsorted(d, d[j - 1], side="left")) \
                if d[j] == d[j - 1] else j
            if j <= i:  # single dst larger than a group (impossible-ish)
                j = min(i + SLOTS_PER_GROUP, n)
        out.append((i, int(j)))
        i = int(j)
    return out


def _preprocess(g, x, W1, b1, W2, b2, W_dgi, edge_index, perm):
    """Build per-core device inputs. Integer index work + dtype staging only."""
    row = np.asarray(edge_index[0], dtype=np.int64)
    col = np.asarray(edge_index[1], dtype=np.int64)
    perm = np.asarray(perm, dtype=np.int64)
    N = g.nreal

    deg = np.bincount(col, minlength=N).astype(np.int64) + 1  # ref: in-deg + 1

    # fused bf16 feature rows, padded to xrows
    X2 = np.zeros((g.xrows, DF), dtype=BF16)
    X2[:N, :D] = x.astype(BF16)
    X2[:N, D:] = x[perm].astype(BF16)

    core_of = col // g.npc
    r2 = (row // g.npc) * g.ldim + (row % g.npc)  # row id in R2-space

    # pass 1: per (core, layer, section) sorted groups
    per_core = []
    for k in range(C):
        m = core_of == k
        rk = row[m]
        dk = col[m] - k * g.npc
        r2k = r2[m]
        layers = []
        for srcid in (rk, r2k):
            sec = srcid // g.sec
            order = np.lexsort((srcid, dk, sec))
            ss, ds_, rs = srcid[order], dk[order], rk[order]
            sec_s = sec[order]
            sections = []
            for s in range(NSEC):
                lo = np.searchsorted(sec_s, s, side="left")
                hi = np.searchsorted(sec_s, s, side="right")
                dloc = ds_[lo:hi]
                groups = _group_walk(dloc)
                sections.append((lo, hi, groups))
            layers.append((ss, ds_, rs, sections))
        per_core.append(layers)

    # uniform calls per (layer, section) across cores
    calls = []
    for li in range(2):
        cl = []
        for s in range(NSEC):
            mx = 0
            for k in range(C):
                ngr = len(per_core[k][li][3][s][2])
                mx = max(mx, -(-ngr // GROUPS_PER_CALL))
            cl.append(mx)
        calls.append(tuple(cl))

    # pass 2: build padded per-core arrays
    deg_f = deg.astype(np.float64)
    ins = []
    for k in range(C):
        d_in = {}
        for li in range(2):
            ss, ds_, rs, sections = per_core[k][li]
            ctot = sum(calls[li])
            S = ctot * SLOTS_PER_CALL
            G = ctot * GROUPS_PER_CALL
            idx = np.zeros(S, dtype=np.int16)
            dstl = np.zeros(S, dtype=np.int32)
            degp = np.full(S, PAD_DEG, dtype=F32)
            # scatter row targets: group result row j -> base+j (real dsts)
            # or a dump row (>= ldim) for unused rows; dump collisions only
            # ever receive zero-adds or races on garbage.
            scidx = np.empty(G * 128, dtype=np.int16)
            jj = np.arange(128)
            goff = 0
            for s in range(NSEC):
                lo, hi, groups = sections[s]
                for gi in range(calls[li][s] * GROUPS_PER_CALL):
                    grow = (goff + gi) * 128
                    if gi < len(groups):
                        a, b = groups[gi]
                        gslot = (goff + gi) * SLOTS_PER_GROUP
                        n = b - a
                        base = int(ds_[lo + a])
                        ndst = int(ds_[lo + b - 1]) - base + 1
                        sl = slice(gslot, gslot + n)
                        idx[sl] = (ss[lo + a : lo + b] - s * g.sec).astype(
                            np.int16
                        )
                        dstl[sl] = ds_[lo + a : lo + b] - base
                        degp[sl] = (
                            deg_f[rs[lo + a : lo + b]]
                            * deg_f[k * g.npc + ds_[lo + a : lo + b]]
                        ).astype(F32)
                        rows = (base + jj).copy()
                        rows[ndst:] = g.ldim + jj[ndst:]
                        scidx[grow : grow + 128] = rows.astype(np.int16)
                    else:
                        scidx[grow : grow + 128] = (g.ldim + jj).astype(
                            np.int16
                        )
                goff += calls[li][s] * GROUPS_PER_CALL
            assert dstl.min() >= 0 and dstl.max() < 128
            # wrapped int16 index layout: slot j -> [j%16, j//16], replicated x8
            idx_w = np.tile(idx.reshape(-1, 16).T, (8, 1)).astype(np.int16)
            sc_w = np.tile(scidx.reshape(-1, 16).T, (8, 1)).astype(np.int16)
            dstl_w = np.ascontiguousarray(
                dstl.reshape(-1, 128).T.astype(F32)
            )
            degp_w = np.ascontiguousarray(degp.reshape(-1, 128).T)
            L = li + 1
            d_in[f"idx{L}"] = np.ascontiguousarray(idx_w)
            d_in[f"scidx{L}"] = np.ascontiguousarray(sc_w)
            d_in[f"dstl{L}"] = dstl_w
            d_in[f"degp{L}"] = degp_w

        # dst-side degree (partition-major [128, nt]) for the self-loop 1/deg
        dd = np.full(g.ldim, PAD_DEG, dtype=F32)
        dd[: g.npc] = deg_f[k * g.npc : (k + 1) * g.npc].astype(F32)
        d_in["degdst"] = np.ascontiguousarray(dd.reshape(g.nt, 128).T)

        # valid-dst mask [128, nt]
        mk = (np.arange(g.ldim) < g.npc).astype(F32)
        d_in["mask"] = np.ascontiguousarray(mk.reshape(g.nt, 128).T)

        d_in["xself"] = np.ascontiguousarray(
            X2[k * g.npc : k * g.npc + g.ldim]
        )
        ins.append(d_in)

    # shared constants
    iota = np.tile(np.arange(128, dtype=F32), (128, 1)).astype(BF16)
    ident = np.eye(128, dtype=BF16)
    wc1 = np.zeros((DF, DF), dtype=F32)
    wc1[:D, :D] = W1
    wc1[D:, D:] = W1
    wc2 = np.zeros((DF, DF), dtype=F32)
    wc2[:D, :D] = W2
    wc2[D:, D:] = W2
    bc1 = np.concatenate([b1, b1]).astype(F32).reshape(DF, 1)
    bc2 = np.concatenate([b2, b2]).astype(F32).reshape(DF, 1)
    # ws2 = (wdgi_stack^T @ summary) * colmask ; wdgi_stack[k, p] = W_dgi[p%64, k]
    wstack = np.zeros((D, DF), dtype=F32)
    wstack[:, :D] = W_dgi.T
    wstack[:, D:] = W_dgi.T
    colmask = np.zeros((DF, 2), dtype=F32)
    colmask[:D, 0] = 1.0
    colmask[D:, 1] = 1.0
    # last-tile pad zeroing pattern, [128,128] along free dim
    nvalid_last = g.npc - (g.nt - 1) * 128
    lastmask = np.tile(
        (np.arange(128) < nvalid_last).astype(F32), (128, 1)
    )
    shared = {
        "x2": X2,
        "iota": iota,
        "ident": ident,
        "identf": np.eye(128, dtype=F32),
        "wc1": wc1,
        "wc2": wc2,
        "bc1": bc1,
        "bc2": bc2,
        "wstack": wstack,
        "colmask": colmask,
        "lastmask": lastmask,
        "ones": np.ones((128, 1), dtype=F32),
    }
    for d_in in ins:
        d_in.update(shared)
    return ins, calls


def _build(g, calls1, calls2):
    dt = mybir.dt
    nc = bacc.Bacc(
        "TRN2", target_bir_lowering=False, debug=False, num_devices=C
    )

    def din(name, shape, dty):
        return nc.dram_tensor(name, list(shape), dty, kind="ExternalInput").ap()

    ct1 = sum(calls1)
    ct2 = sum(calls2)
    x2 = din("x2", (g.xrows, DF), dt.bfloat16)
    xself = din("xself", (g.ldim, DF), dt.bfloat16)
    idx_d = [
        din("idx1", (128, ct1 * 256), dt.int16),
        din("idx2", (128, ct2 * 256), dt.int16),
    ]
    dstl_d = [
        din("dstl1", (128, ct1 * TILES_PER_CALL), dt.float32),
        din("dstl2", (128, ct2 * TILES_PER_CALL), dt.float32),
    ]
    degp_d = [
        din("degp1", (128, ct1 * TILES_PER_CALL), dt.float32),
        din("degp2", (128, ct2 * TILES_PER_CALL), dt.float32),
    ]
    scidx_d = [
        din("scidx1", (128, ct1 * GROUPS_PER_CALL * 8), dt.int16),
        din("scidx2", (128, ct2 * GROUPS_PER_CALL * 8), dt.int16),
    ]
    identf_d = din("identf", (128, 128), dt.float32)
    degdst = din("degdst", (128, g.nt), dt.float32)
    mask_d = din("mask", (128, g.nt), dt.float32)
    iota_d = din("iota", (128, 128), dt.bfloat16)
    ident_d = din("ident", (128, 128), dt.bfloat16)
    wc_d = [din("wc1", (DF, DF), dt.float32), din("wc2", (DF, DF), dt.float32)]
    bc_d = [din("bc1", (DF, 1), dt.float32), din("bc2", (DF, 1), dt.float32)]
    wstack_d = din("wstack", (D, DF), dt.float32)
    colmask_d = din("colmask", (DF, 2), dt.float32)
    lastmask_d = din("lastmask", (128, 128), dt.float32)
    ones_d = din("ones", (128, 1), dt.float32)
    loss_out = nc.dram_tensor("loss", [1, 16], dt.float32, kind="ExternalOutput").ap()

    inv_n = 1.0 / float(g.nreal)
    rg = [list(range(C))]

    with tile.TileContext(nc) as tc, ExitStack() as ctx:
        dram = ctx.enter_context(tc.tile_pool(name="dram", bufs=1, space="DRAM"))
        r2shard = dram.tile([g.ldim, DF], dt.bfloat16, tag="r2shard")
        r2full = dram.tile([g.xrows, DF], dt.bfloat16, tag="r2full", addr_space="Shared")
        cs_in = dram.tile([128, 1], dt.float32, tag="cs_in")
        cs_out = dram.tile([128, 1], dt.float32, tag="cs_out", addr_space="Shared")
        ls_in = dram.tile([1, 16], dt.float32, tag="ls_in")
        ls_out = dram.tile([1, 16], dt.float32, tag="ls_out", addr_space="Shared")

        const = ctx.enter_context(tc.tile_pool(name="const", bufs=1))

        def cload(ap_dram, shape, dty, tag):
            t = const.tile(list(shape), dty, tag=tag)
            nc.sync.dma_start(t[:], ap_dram)
            return t

        iota_sb = cload(iota_d, (128, 128), dt.bfloat16, "iota")
        ident_sb = cload(ident_d, (128, 128), dt.bfloat16, "ident")
        identf_sb = cload(identf_d, (128, 128), dt.float32, "identf")
        wc_sb = [
            cload(wc_d[0], (DF, DF), dt.float32, "wc1"),
            cload(wc_d[1], (DF, DF), dt.float32, "wc2"),
        ]
        bc_sb = [
            cload(bc_d[0], (DF, 1), dt.float32, "bc1"),
            cload(bc_d[1], (DF, 1), dt.float32, "bc2"),
        ]
        wstack_sb = cload(wstack_d, (D, DF), dt.float32, "wstack")
        colmask_sb = cload(colmask_d, (DF, 2), dt.float32, "colmask")
        lastmask_sb = cload(lastmask_d, (128, 128), dt.float32, "lastmask")
        ones_sb = cload(ones_d, (128, 1), dt.float32, "ones")
        mask_sb = cload(mask_d, (128, g.nt), dt.float32, "mask")

        big = ctx.enter_context(tc.tile_pool(name="big", bufs=1))
        z_sb = big.tile([128, g.ldim], dt.float32, tag="z_sb")
        # DRAM accumulator, d-major: rows [0, ldim) real dsts, rows
        # [ldim, ldim+128) dump for unused scatter rows.
        accr = g.ldim + 128
        zrows = 128 * max(1, min(10, accr // 128))
        while accr % zrows != 0:
            zrows -= 128
        acc_dram = dram.tile([accr, DF], dt.float32, tag="acc_dram")
        zt = big.tile([128, (zrows // 128) * DF], dt.float32, tag="zt")
        nc.vector.memset(zt[:], 0.0)

        # slot-norm metadata pool; tags shared between layers (sequential use)
        meta = ctx.enter_context(tc.tile_pool(name="meta", bufs=1))

        # dst self-loop scale: 1/deg  [128, nt]
        dd = meta.tile([128, g.nt], dt.float32, tag="dd")
        nc.sync.dma_start(dd[:], degdst)
        dis2 = meta.tile([128, g.nt], dt.float32, tag="dis2")
        nc.vector.reciprocal(dis2[:], dd[:])

        stg = ctx.enter_context(tc.tile_pool(name="stg", bufs=2))
        idxp = ctx.enter_context(tc.tile_pool(name="idxp", bufs=3))
        gpool = ctx.enter_context(tc.tile_pool(name="gpool", bufs=3))
        ppool = ctx.enter_context(tc.tile_pool(name="ppool", bufs=6))
        psg = ctx.enter_context(tc.tile_pool(name="psg", bufs=3, space="PSUM"))
        pst = ctx.enter_context(tc.tile_pool(name="pst", bufs=2, space="PSUM"))
        psm = ctx.enter_context(tc.tile_pool(name="psm", bufs=2, space="PSUM"))
        psl = ctx.enter_context(tc.tile_pool(name="psl", bufs=1, space="PSUM"))
        work = ctx.enter_context(tc.tile_pool(name="work", bufs=3))
        outp = ctx.enter_context(tc.tile_pool(name="outp", bufs=3))

        def aggregate(li, calls, src_dram):
            ct = sum(calls)
            dg = meta.tile([128, ct * TILES_PER_CALL], dt.float32, tag="dg")
            nc.sync.dma_start(dg[:], degp_d[li])
            nc.vector.reciprocal(dg[:], dg[:])
            wv = meta.tile([128, ct * TILES_PER_CALL], dt.float32, tag="wv")
            nc.scalar.sqrt(wv[:], dg[:])
            dl = meta.tile([128, ct * TILES_PER_CALL], dt.float32, tag="dl")
            nc.sync.dma_start(dl[:], dstl_d[li])
            # zero the accumulator
            nz = accr // zrows
            for zi in range(nz):
                nc.sync.dma_start(
                    acc_dram[zi * zrows : (zi + 1) * zrows, :].rearrange(
                        "(p a) f -> p (a f)", p=128
                    ),
                    zt[:],
                )
            cglob = 0
            for s in range(NSEC):
                src_sec = src_dram[s * g.sec : (s + 1) * g.sec, :]
                # scatter batches of <= SCB calls; groups within one section
                # have disjoint dsts, so batch scatters of the same section
                # cannot race each other.
                nbat = -(-calls[s] // SCB)
                batches = []
                c0 = 0
                for _b in range(nbat):
                    c1 = min(c0 + SCB, calls[s])
                    batches.append((c0, c1))
                    c0 = c1
                for (c0, c1) in batches:
                  nsg = (c1 - c0) * GROUPS_PER_CALL
                  stage = stg.tile(
                      [128, SCB * GROUPS_PER_CALL, DF], dt.float32, tag="stage"
                  )
                  sci = idxp.tile(
                      [128, SCB * GROUPS_PER_CALL * 8], dt.int16, tag="sci"
                  )
                  nc.sync.dma_start(
                      sci[:, : nsg * 8],
                      scidx_d[li][
                          :, cglob * GROUPS_PER_CALL * 8 :
                          (cglob * GROUPS_PER_CALL + nsg) * 8
                      ],
                  )
                  g0 = cglob * GROUPS_PER_CALL
                  for _c in range(c1 - c0):
                      it = idxp.tile([128, 256], dt.int16, tag="it")
                      nc.sync.dma_start(
                          it[:], idx_d[li][:, cglob * 256 : (cglob + 1) * 256]
                      )
                      gt = gpool.tile(
                          [128, TILES_PER_CALL, DF], dt.bfloat16, tag="gt"
                      )
                      nc.gpsimd.dma_gather(
                          gt[:], src_sec, it[:], SLOTS_PER_CALL, SLOTS_PER_CALL,
                          DF, single_packet=False,
                      )
                      for q in range(GROUPS_PER_CALL):
                          ps = psg.tile([128, 128], dt.float32, tag="ps")
                          for t in range(TILES_PER_GROUP):
                              tc_i = q * TILES_PER_GROUP + t
                              col = cglob * TILES_PER_CALL + tc_i
                              P = ppool.tile([128, 128], dt.bfloat16, tag="P")
                              nc.vector.tensor_scalar(
                                  P[:],
                                  iota_sb[:],
                                  dl[:, col : col + 1],
                                  wv[:, col : col + 1],
                                  mybir.AluOpType.is_equal,
                                  mybir.AluOpType.mult,
                              )
                              nc.tensor.matmul(
                                  ps[:],
                                  lhsT=P[:],
                                  rhs=gt[:, tc_i, :],
                                  start=(t == 0),
                                  stop=(t == TILES_PER_GROUP - 1),
                              )
                          gl = cglob * GROUPS_PER_CALL + q - g0
                          nc.vector.tensor_copy(stage[:, gl, :], ps[:])
                      cglob += 1
                  nc.gpsimd.dma_scatter_add(
                      acc_dram[:],
                      stage[:, :nsg, :],
                      sci[:, : nsg * 8],
                      nsg * 128,
                      nsg * 128,
                      DF,
                      single_packet=False,
                  )

        def post(li, self_src, to_r2):
            # u[d,f] = acc[d,:] + (1/deg_d)*xself[d,:]; out = wc^T @ u^T; +b
            for dti in range(g.nt):
                sl = slice(dti * 128, (dti + 1) * 128)
                at = work.tile([128, DF], dt.float32, tag="at")
                nc.sync.dma_start(at[:], acc_dram[sl, :])
                xs = work.tile([128, DF], dt.bfloat16, tag="xs")
                nc.sync.dma_start(xs[:], self_src[sl, :])
                xss = work.tile([128, DF], dt.bfloat16, tag="xss")
                nc.vector.tensor_scalar(
                    xss[:],
                    xs[:],
                    dis2[:, dti : dti + 1],
                    None,
                    mybir.AluOpType.mult,
                )
                u = work.tile([128, 128], dt.float32, tag="u")
                nc.vector.tensor_tensor(
                    u[:], at[:], xss[:], op=mybir.AluOpType.add
                )
                tp = pst.tile([128, 128], dt.float32, tag="tp")
                nc.tensor.transpose(tp[:], u[:], identf_sb[:])
                ut = work.tile([128, 128], dt.float32, tag="ut")
                nc.vector.tensor_copy(ut[:], tp[:])
                po = psm.tile([128, 128], dt.float32, tag="po")
                nc.tensor.matmul(
                    po[:], lhsT=wc_sb[li][:], rhs=ut[:], start=True, stop=True
                )
                if to_r2:
                    rb = outp.tile([128, 128], dt.bfloat16, tag="rb")
                    nc.vector.tensor_scalar(
                        rb[:],
                        po[:],
                        bc_sb[li][:],
                        0.0,
                        mybir.AluOpType.add,
                        mybir.AluOpType.max,
                    )
                    rt = outp.tile([128, 128], dt.bfloat16, tag="rt")
                    nc.sync.dma_start_transpose(rt[:], rb[:])
                    nc.sync.dma_start(r2shard[sl, :], rt[:])
                else:
                    nc.vector.tensor_scalar(
                        z_sb[:, sl],
                        po[:],
                        bc_sb[li][:],
                        None,
                        mybir.AluOpType.add,
                    )
                    if dti == g.nt - 1:
                        nc.vector.tensor_tensor(
                            z_sb[:, sl],
                            z_sb[:, sl],
                            lastmask_sb[:],
                            op=mybir.AluOpType.mult,
                        )

        aggregate(0, calls1, x2)
        post(0, xself, to_r2=True)

        nc.gpsimd.collective_compute(
            "AllGather",
            mybir.AluOpType.bypass,
            replica_groups=rg,
            ins=[r2shard[:].opt()],
            outs=[r2full[:].opt()],
        )

        aggregate(1, calls2, r2full[:])
        post(1, r2shard[:], to_r2=False)

        # ---- DGI readout ----
        fin = ctx.enter_context(tc.tile_pool(name="fin", bufs=1))
        cs = fin.tile([128, 1], dt.float32, tag="cs")
        nc.vector.reduce_sum(cs[:], z_sb[:], axis=mybir.AxisListType.X)
        nc.sync.dma_start(cs_in[:], cs[:])
        nc.gpsimd.collective_compute(
            "AllReduce",
            mybir.AluOpType.add,
            replica_groups=rg,
            ins=[cs_in[:].opt()],
            outs=[cs_out[:].opt()],
        )
        cst = fin.tile([128, 1], dt.float32, tag="cst")
        nc.sync.dma_start(cst[:], cs_out[:])
        summ = fin.tile([128, 1], dt.float32, tag="summ")
        nc.scalar.activation(
            summ[:], cst[:], mybir.ActivationFunctionType.Sigmoid, scale=inv_n
        )
        wsps = psl.tile([DF, 1], dt.float32, tag="pls")
        nc.tensor.matmul(
            wsps[:], lhsT=wstack_sb[:], rhs=summ[0:D, 0:1], start=True, stop=True
        )
        ws2 = fin.tile([DF, 2], dt.float32, tag="ws2")
        nc.vector.tensor_tensor(
            ws2[:],
            colmask_sb[:],
            wsps[:].to_broadcast([DF, 2]),
            op=mybir.AluOpType.mult,
        )
        tp_sb = fin.tile([128, g.nt], dt.float32, tag="tp_sb")
        tn_sb = fin.tile([128, g.nt], dt.float32, tag="tn_sb")
        for dti in range(g.nt):
            sl = slice(dti * 128, (dti + 1) * 128)
            tps = psl.tile([128, 2], dt.float32, tag="pls")
            nc.tensor.matmul(
                tps[:], lhsT=z_sb[:, sl], rhs=ws2[:], start=True, stop=True
            )
            nc.vector.tensor_copy(tp_sb[:, dti : dti + 1], tps[:, 0:1])
            nc.vector.tensor_copy(tn_sb[:, dti : dti + 1], tps[:, 1:2])

        # softplus(sgn*t) = relu(sgn*t) + ln1p(exp(-|t|)); deg-7 poly for ln1p
        LN1P = [
            5.62195900721818e-07, 0.9999574870750696, -0.4992065685478763,
            0.32697310001391783, -0.2228362583278401, 0.13076503250360005,
            -0.05262485136716543, 0.010119082927575069,
        ]

        def softplus_of(t_in, sgn, tagp):
            neg = fin.tile([128, g.nt], dt.float32, tag=f"{tagp}neg")
            nc.vector.tensor_scalar(
                neg[:], t_in[:], -1.0, None, mybir.AluOpType.mult
            )
            ab = fin.tile([128, g.nt], dt.float32, tag=f"{tagp}ab")
            nc.vector.tensor_tensor(ab[:], t_in[:], neg[:], op=mybir.AluOpType.max)
            uu = fin.tile([128, g.nt], dt.float32, tag=f"{tagp}uu")
            nc.scalar.activation(
                uu[:], ab[:], mybir.ActivationFunctionType.Exp, scale=-1.0
            )
            pp_ = fin.tile([128, g.nt], dt.float32, tag=f"{tagp}pp")
            nc.vector.tensor_scalar(
                pp_[:], uu[:], LN1P[7], LN1P[6],
                mybir.AluOpType.mult, mybir.AluOpType.add,
            )
            pm = fin.tile([128, g.nt], dt.float32, tag=f"{tagp}pm")
            for ci in range(5, -1, -1):
                nc.vector.tensor_tensor(
                    pm[:], pp_[:], uu[:], op=mybir.AluOpType.mult
                )
                nc.vector.tensor_scalar(
                    pp_[:], pm[:], LN1P[ci], None, mybir.AluOpType.add
                )
            rl = fin.tile([128, g.nt], dt.float32, tag=f"{tagp}rl")
            nc.vector.tensor_scalar(
                rl[:], (t_in if sgn > 0 else neg)[:], 0.0, None,
                mybir.AluOpType.max,
            )
            res = fin.tile([128, g.nt], dt.float32, tag=f"{tagp}res")
            nc.vector.tensor_tensor(res[:], rl[:], pp_[:], op=mybir.AluOpType.add)
            return res

        spp = softplus_of(tp_sb, -1, "sp")   # softplus(-t_pos)
        spn = softplus_of(tn_sb, +1, "sn")   # softplus(t_neg)
        ssum = fin.tile([128, g.nt], dt.float32, tag="ssum")
        nc.vector.tensor_tensor(ssum[:], spp[:], spn[:], op=mybir.AluOpType.add)
        nc.vector.tensor_tensor(
            ssum[:], ssum[:], mask_sb[:], op=mybir.AluOpType.mult
        )
        srow = fin.tile([128, 1], dt.float32, tag="srow")
        nc.vector.reduce_sum(srow[:], ssum[:], axis=mybir.AxisListType.X)
        tot = psl.tile([1, 1], dt.float32, tag="pls")
        nc.tensor.matmul(
            tot[:], lhsT=srow[:], rhs=ones_sb[:], start=True, stop=True
        )
        lsb = fin.tile([1, 16], dt.float32, tag="lsb")
        nc.vector.memset(lsb[:], 0.0)
        nc.vector.tensor_copy(lsb[0:1, 0:1], tot[:])
        nc.sync.dma_start(ls_in[:], lsb[:])
        nc.gpsimd.collective_compute(
            "AllReduce",
            mybir.AluOpType.add,
            replica_groups=rg,
            ins=[ls_in[:].opt()],
            outs=[ls_out[:].opt()],
        )
        lsf = fin.tile([1, 16], dt.float32, tag="lsf")
        nc.sync.dma_start(lsf[:], ls_out[:])
        lout = fin.tile([1, 16], dt.float32, tag="lout")
        nc.scalar.activation(
            lout[:], lsf[:], mybir.ActivationFunctionType.Copy, scale=inv_n
        )
        nc.sync.dma_start(loss_out, lout[:])

    nc.compile()
    return nc


_prog_cache = {}


def _get_prog(g, calls1, calls2):
    key = (g.npc, g.nreal, calls1, calls2)
    if key not in _prog_cache:
        _prog_cache[key] = _build(g, calls1, calls2)
    return _prog_cache[key]


def run(inputs, npc, nreal, trace=False):
    g = Geo(npc, nreal)
    in_maps, calls = _preprocess(g, **inputs)
    nc = _get_prog(g, calls[0], calls[1])
    res = run_bass_kernel_spmd(
        nc, in_maps, core_ids=list(range(C)), trace=trace
    )
    loss = res.results[0]["loss"][0, 0]
    return np.float32(loss), res


def kernel(**inputs):
    out, _ = run(inputs, npc=12500, nreal=100000)
    return out


def _make_sharded_exec(nc, in_maps, reps=1):
    """Reusable jitted shard_map executor mirroring bass2jax's multi-core
    path, with device-resident inputs. With reps>1 the NEFF is executed
    reps times inside one dispatch so per-execution time can be resolved
    above the ~200ms axon dispatch floor."""
    import jax
    from jax.experimental.shard_map import shard_map
    from jax.sharding import Mesh, NamedSharding, PartitionSpec

    from concourse import bass2jax, mybir as _mb

    bass2jax.install_neuronx_cc_hook()
    partition_name = (
        nc.partition_id_tensor.name if nc.partition_id_tensor else None
    )
    in_names, out_names, out_avals, zero_shapes = [], [], [], []
    for alloc in nc.m.functions[0].allocations:
        if not isinstance(alloc, _mb.MemoryLocationSet):
            continue
        name = alloc.memorylocations[0].name
        if alloc.kind == "ExternalInput":
            if name != partition_name:
                in_names.append(name)
        elif alloc.kind == "ExternalOutput":
            shape = tuple(alloc.tensor_shape)
            dty = _mb.dt.np(alloc.dtype)
            out_names.append(name)
            out_avals.append(jax.core.ShapedArray(shape, dty))
            zero_shapes.append((shape, dty))
    n_params = len(in_names)
    n_outs = len(out_avals)
    all_names = list(in_names) + list(out_names)
    if partition_name is not None:
        all_names.append(partition_name)
    donate = tuple(range(n_params, n_params + n_outs * reps))

    assert reps == 1  # the neuronx_cc hook allows one bass_exec per module

    def _body(*args):
        operands = list(args)
        if partition_name is not None:
            operands.append(bass2jax.partition_id_tensor())
        outs = bass2jax._bass_exec_p.bind(
            *operands,
            out_avals=tuple(out_avals),
            in_names=tuple(all_names),
            out_names=tuple(out_names),
            lowering_input_output_aliases=(),
            sim_require_finite=True,
            sim_require_nnan=True,
            nc=nc,
        )
        return tuple(outs)

    devices = jax.devices()[:C]
    mesh = Mesh(np.array(devices), ("core",))
    spec = PartitionSpec("core")
    sharded = jax.jit(
        shard_map(
            _body,
            mesh=mesh,
            in_specs=(spec,) * (n_params + n_outs * reps),
            out_specs=(spec,) * n_outs,
            check_rep=False,
        ),
        donate_argnums=donate,
        keep_unused=True,
    )
    shard = NamedSharding(mesh, spec)
    concat_in = [
        jax.device_put(
            np.concatenate([np.asarray(m[nm]) for m in in_maps], axis=0), shard
        )
        for nm in in_names
    ]

    def launch():
        zeros = [
            jax.device_put(np.zeros((C * s[0], *s[1:]), d), shard)
            for (s, d) in zero_shapes
        ]
        return sharded(*concat_in, *zeros)

    def fetch(outs):
        jax.block_until_ready(outs)
        return {
            nm: np.asarray(outs[i]).reshape(C, *out_avals[i].shape)[0]
            for i, nm in enumerate(out_names)
        }

    def run_once():
        return fetch(launch())

    run_once.launch = launch
    run_once.fetch = fetch
    return run_once


def bench(inputs, npc=12500, nreal=100000, iters=6):
    import time

    g = Geo(npc, nreal)
    t0 = time.time()
    in_maps, calls = _preprocess(g, **inputs)
    t1 = time.time()
    nc = _get_prog(g, calls[0], calls[1])
    t2 = time.time()
    run_1 = _make_sharded_exec(nc, in_maps)
    out = run_1()  # warmup: compiles + loads NEFF
    t3 = time.time()
    t1s = []
    for _ in range(iters):
        ta = time.time()
        out = run_1()
        t1s.append(time.time() - ta)
    # pipelined async launches: marginal cost per launch approximates
    # NEFF execution + per-exec overhead without the full dispatch floor
    K = 48
    ta = time.time()
    pend = [run_1.launch() for _ in range(K)]
    import jax as _jax
    _jax.block_until_ready(pend)
    tK = time.time() - ta
    per = (tK - min(t1s)) / (K - 1)
    print(
        f"preprocess {t1-t0:.1f}s  build {t2-t1:.1f}s  warmup {t3-t2:.1f}s\n"
        f"  1-shot ms: {[round(t*1e3,2) for t in t1s]}\n"
        f"  {K} pipelined: total {tK*1e3:.1f} ms -> marginal {per*1e3:.3f} ms"
    )
    return np.float32(out["loss"][0, 0]), per



# revision 23
# speedup vs baseline: 1.8600x; 1.1943x over previous
"""DeepGraphInfomax loss (2-layer GCN encoder, pos+neg, DGI readout) on 8 trn2 cores.

Window-major dst-sharded pull-mode GNN aggregation:
  - Nodes (dst rows) sharded contiguously across 8 cores (12500 each).
  - pos/neg feature streams fused into 128-wide rows: X2[r] = [x[r] | x[perm[r]]].
  - Self-loops folded in as explicit edges with degree product deg^2, so the
    aggregation produces the complete GCN pre-activation in one pass.
  - Source rows live in a quarter-major layout: node (core k, local l) maps to
    row 25600*(l//3200) + 3200*k + (l%3200).  The 4 sections of 25600 rows keep
    int16 gather indices valid, AND layer-1 (x2q) and layer-2 (r2full) share
    the exact same index space, so idx/dstl/norm arrays are staged and loaded
    once for both layers.
  - Processing is window-major: all tiles of one 128-dst window (across all 4
    source sections) accumulate into a single PSUM tile via one-hot matmuls
    with swapped operands (lhsT=gathered rows, rhs=one-hot), yielding
    feature-major results directly.  No DRAM accumulator, no scatter-add.
  - post per window: PE applies W (A @ (X W) == (A @ X) W) straight from the
    SBUF accumulator, DVE applies bias(+relu); layer-1 results are transposed
    on the PE (not the DMA xbar: Tile serializes DMA-transposes with in-flight
    collectives) and stored row-major bf16 to r2shard.
  - r2shard is AllGathered in 4 quarter chunks, each gated only on the quarter
    of post-L1 windows it needs, so layer-2 gathers start while layer-1 post
    is still finishing.
  - DGI readout (summary / W_dgi / softplus losses) computed on device with two
    tiny AllReduces.

Host-side preprocessing only manipulates integer graph structure (sorting,
degree counts, packing, index mapping) and stages dtype-cast copies of the
inputs; all floating-point math of the reference runs on device.
"""

import sys

for _p in ("/opt/trn_rl_repo", "/root/.axon_site/_ro/trn_rl_repo"):
    if _p not in sys.path:
        sys.path.insert(0, _p)

from contextlib import ExitStack

import ml_dtypes
import numpy as np

import concourse.bass as bass
import concourse.bacc as bacc
import concourse.mybir as mybir
import concourse.tile as tile
from concourse.bass_utils import run_bass_kernel_spmd

BF16 = ml_dtypes.bfloat16
F32 = np.float32

C = 8            # cores
D = 64           # hidden dim
DF = 2 * D       # fused pos|neg width
NSEC = 4
NAG = 4          # AllGather chunks (1 or NSEC)
TILES_PER_CALL = 32
SLOTS_PER_CALL = TILES_PER_CALL * 128
PAD_DEG = 1e30   # pad-slot degree product -> norm ~ 1e-15 ~ 0


class Geo:
    def __init__(self, npc, nreal):
        self.npc = npc                       # real nodes per core
        self.nreal = nreal                   # total real nodes (= 8*npc)
        self.nw = -(-npc // 128)             # dst windows per core (98)
        self.ldim = 128 * self.nw            # padded dsts per core (12544)
        self.wpq = -(-self.nw // NSEC)       # windows per quarter (25)
        self.ql = self.wpq * 128             # locals per quarter (3200)
        self.sec = C * self.ql               # rows per section (25600)
        self.xrows = NSEC * self.sec         # padded source-row space (102400)
        self.shard = NSEC * self.ql          # r2shard rows (12800)
        assert self.sec < 32768


def _preprocess(g, x, W1, b1, W2, b2, W_dgi, edge_index, perm):
    """Build per-core device inputs. Integer index work + dtype staging only."""
    row = np.asarray(edge_index[0], dtype=np.int64)
    col = np.asarray(edge_index[1], dtype=np.int64)
    perm = np.asarray(perm, dtype=np.int64)
    N = g.nreal
    npc, ql = g.npc, g.ql

    deg = np.bincount(col, minlength=N).astype(np.int64) + 1  # in-deg + 1

    # source-row id per global node: quarter-major for NAG=4 (chunked
    # AllGathers concat per-quarter), core-major for NAG=1 (single AllGather
    # concatenates full shards)
    gids = np.arange(N, dtype=np.int64)
    kk = gids // npc
    ll = gids % npc
    if NAG == 1:
        r2p = g.shard * kk + ll
    else:
        r2p = g.sec * (ll // ql) + ql * kk + (ll % ql)

    # fused bf16 feature rows in quarter-major layout
    X2 = np.zeros((g.xrows, DF), dtype=BF16)
    X2[r2p, :D] = x.astype(BF16)
    X2[r2p, D:] = x[perm].astype(BF16)

    # edges + self-loops (self: src == dst, degp = deg^2 -> weight 1/deg)
    rows_a = np.concatenate([row, gids])
    cols_a = np.concatenate([col, gids])
    src_q = r2p[rows_a]                       # quarter-major src row
    kd = cols_a // npc                        # dst core
    dl = cols_a % npc                         # dst local
    sec = src_q // g.sec
    w = dl // 128

    # tile counts per (core, sec, window) -> T = max over cores
    key = ((kd * NSEC + sec) * g.nw + w).astype(np.int64)
    cnt = np.bincount(key, minlength=C * NSEC * g.nw).reshape(C, NSEC, g.nw)
    T = np.maximum(-(-cnt // 128), 0).max(axis=0)           # [NSEC, NW]
    tiles_s = T.sum(axis=1)                                 # tiles per section
    ntiles = int(tiles_s.sum())
    calls = [
        [
            TILES_PER_CALL
            if (c + 1) * TILES_PER_CALL <= tiles_s[s]
            else int(tiles_s[s] - c * TILES_PER_CALL)
            for c in range(-(-int(tiles_s[s]) // TILES_PER_CALL))
        ]
        for s in range(NSEC)
    ]
    tbase = np.concatenate([[0], np.cumsum(tiles_s)])       # section tile base
    # slot base of each (s, w) run
    wbase = np.zeros((NSEC, g.nw), dtype=np.int64)
    for s in range(NSEC):
        wbase[s] = (tbase[s] + np.concatenate([[0], np.cumsum(T[s])[:-1]])) * 128

    deg_f = deg.astype(np.float64)
    degp_a = deg_f[rows_a] * deg_f[cols_a]

    ins = []
    for k in range(C):
        m = kd == k
        sq, dk, wk, sk = src_q[m], dl[m], w[m], sec[m]
        dp = degp_a[m]
        order = np.lexsort((sq, dk, wk, sk))
        sq, dk, wk, sk, dp = (a[order] for a in (sq, dk, wk, sk, dp))
        # rank within each (sec, window) run
        runkey = sk * g.nw + wk
        starts = np.searchsorted(runkey, runkey, side="left")
        rank = np.arange(len(runkey)) - starts
        slot = wbase[sk, wk] + rank

        S = ntiles * 128
        idx = np.zeros(S, dtype=np.int16)
        dstl = np.zeros(S, dtype=np.int32)
        degp = np.full(S, PAD_DEG, dtype=F32)
        idx[slot] = (sq - sk * g.sec).astype(np.int16)
        dstl[slot] = dk - wk * 128
        degp[slot] = dp.astype(F32)
        assert dstl.min() >= 0 and dstl.max() < 128

        d_in = {
            # wrapped int16 index layout: slot j -> [j%16, j//16], replicated x8
            "idx": np.ascontiguousarray(
                np.tile(idx.reshape(-1, 16).T, (8, 1)).astype(np.int16)
            ),
            "dstl": np.ascontiguousarray(dstl.reshape(-1, 128).T.astype(F32)),
            "degp": np.ascontiguousarray(degp.reshape(-1, 128).T),
        }
        ins.append(d_in)

    # shared constants
    iota = np.tile(np.arange(128, dtype=F32), (128, 1)).astype(BF16)
    ident = np.eye(128, dtype=F32).astype(BF16)
    wc1 = np.zeros((DF, DF), dtype=F32)
    wc1[:D, :D] = W1
    wc1[D:, D:] = W1
    wc2 = np.zeros((DF, DF), dtype=F32)
    wc2[:D, :D] = W2
    wc2[D:, D:] = W2
    bc1 = np.concatenate([b1, b1]).astype(F32).reshape(DF, 1)
    bc2 = np.concatenate([b2, b2]).astype(F32).reshape(DF, 1)
    wstack = np.zeros((D, DF), dtype=F32)
    wstack[:, :D] = W_dgi.T
    wstack[:, D:] = W_dgi.T
    colmask = np.zeros((DF, 2), dtype=F32)
    colmask[:D, 0] = 1.0
    colmask[D:, 1] = 1.0
    nvalid_last = g.npc - (g.nw - 1) * 128
    lastmask = np.tile((np.arange(128) < nvalid_last).astype(F32), (128, 1))
    mk = (np.arange(g.ldim) < g.npc).astype(F32)
    shared = {
        "x2": X2,
        "iota": iota,
        "ident": ident,
        "wc1": wc1,
        "wc2": wc2,
        "bc1": bc1,
        "bc2": bc2,
        "wstack": wstack,
        "colmask": colmask,
        "lastmask": lastmask,
        "mask": np.ascontiguousarray(mk.reshape(g.nw, 128).T),
        "ones": np.ones((128, 1), dtype=F32),
    }
    for d_in in ins:
        d_in.update(shared)
    struct = (tuple(map(tuple, T)), tuple(map(tuple, calls)))
    return ins, struct


def _build(g, struct):
    T, calls = struct
    T = [list(r) for r in T]
    calls = [list(r) for r in calls]
    tiles_s = [sum(r) for r in T]
    ntiles = sum(tiles_s)
    tbase = [0]
    for s in range(NSEC):
        tbase.append(tbase[-1] + tiles_s[s])

    dt = mybir.dt
    nc = bacc.Bacc(
        "TRN2", target_bir_lowering=False, debug=False, num_devices=C
    )

    def din(name, shape, dty):
        return nc.dram_tensor(name, list(shape), dty, kind="ExternalInput").ap()

    x2 = din("x2", (g.xrows, DF), dt.bfloat16)
    idx_d = din("idx", (128, ntiles * 8), dt.int16)
    dstl_d = din("dstl", (128, ntiles), dt.float32)
    degp_d = din("degp", (128, ntiles), dt.float32)
    iota_d = din("iota", (128, 128), dt.bfloat16)
    ident_d = din("ident", (128, 128), dt.bfloat16)
    wc_d = [din("wc1", (DF, DF), dt.float32), din("wc2", (DF, DF), dt.float32)]
    bc_d = [din("bc1", (DF, 1), dt.float32), din("bc2", (DF, 1), dt.float32)]
    wstack_d = din("wstack", (D, DF), dt.float32)
    colmask_d = din("colmask", (DF, 2), dt.float32)
    lastmask_d = din("lastmask", (128, 128), dt.float32)
    mask_d = din("mask", (128, g.nw), dt.float32)
    ones_d = din("ones", (128, 1), dt.float32)
    loss_out = nc.dram_tensor("loss", [1, 16], dt.float32, kind="ExternalOutput").ap()

    inv_n = 1.0 / float(g.nreal)
    rg = [list(range(C))]

    with tile.TileContext(nc) as tc, ExitStack() as ctx:
        dram = ctx.enter_context(tc.tile_pool(name="dram", bufs=1, space="DRAM"))
        if NAG == NSEC:
            r2shard = []
            for j in range(NSEC):
                r2s_j = dram.tile(
                    [g.ql, DF], dt.bfloat16, tag=f"r2shard{j}",
                    name=f"r2shard_{j}",
                )
                r2shard.append(r2s_j)
            r2full = []
            for j in range(NSEC):
                r2f_j = dram.tile(
                    [g.sec, DF], dt.bfloat16, tag=f"r2full{j}",
                    addr_space="Shared", name=f"r2full_{j}",
                )
                r2full.append(r2f_j)
        else:
            r2shard_1 = dram.tile(
                [g.shard, DF], dt.bfloat16, tag="r2shard", name="r2shard_1"
            )
            r2full_1 = dram.tile(
                [g.xrows, DF], dt.bfloat16, tag="r2full",
                addr_space="Shared", name="r2full_1",
            )
            r2shard = [
                r2shard_1[j * g.ql : (j + 1) * g.ql, :] for j in range(NSEC)
            ]
            r2full = [
                r2full_1[j * g.sec : (j + 1) * g.sec, :] for j in range(NSEC)
            ]
        cs_in = dram.tile([128, 1], dt.float32, tag="cs_in")
        cs_out = dram.tile([128, 1], dt.float32, tag="cs_out", addr_space="Shared")
        ls_in = dram.tile([1, 16], dt.float32, tag="ls_in")
        ls_out = dram.tile([1, 16], dt.float32, tag="ls_out", addr_space="Shared")

        const = ctx.enter_context(tc.tile_pool(name="const", bufs=1))

        def cload(ap_dram, shape, dty, tag):
            t = const.tile(list(shape), dty, tag=tag)
            nc.sync.dma_start(t[:], ap_dram)
            return t

        iota_sb = cload(iota_d, (128, 128), dt.bfloat16, "iota")
        ident_sb = cload(ident_d, (128, 128), dt.bfloat16, "ident")
        wc_sb = [
            cload(wc_d[0], (DF, DF), dt.float32, "wc1"),
            cload(wc_d[1], (DF, DF), dt.float32, "wc2"),
        ]
        bc_sb = [
            cload(bc_d[0], (DF, 1), dt.float32, "bc1"),
            cload(bc_d[1], (DF, 1), dt.float32, "bc2"),
        ]
        wstack_sb = cload(wstack_d, (D, DF), dt.float32, "wstack")
        colmask_sb = cload(colmask_d, (DF, 2), dt.float32, "colmask")
        lastmask_sb = cload(lastmask_d, (128, 128), dt.float32, "lastmask")
        mask_sb = cload(mask_d, (128, g.nw), dt.float32, "mask")
        ones_sb = cload(ones_d, (128, 1), dt.float32, "ones")

        big = ctx.enter_context(tc.tile_pool(name="big", bufs=1))
        z_sb = big.tile([128, g.ldim], dt.float32, tag="z_sb")
        acc = big.tile([128, g.ldim], dt.float32, tag="acc")
        dl_sb = big.tile([128, ntiles], dt.float32, tag="dl_sb")
        nc.sync.dma_start(dl_sb[:], dstl_d)
        wv_sb = big.tile([128, ntiles], dt.float32, tag="wv_sb")
        nc.sync.dma_start(wv_sb[:], degp_d)
        nc.vector.reciprocal(wv_sb[:], wv_sb[:])
        nc.scalar.sqrt(wv_sb[:], wv_sb[:])

        gpool = ctx.enter_context(tc.tile_pool(name="gpool", bufs=8))
        ipool = ctx.enter_context(tc.tile_pool(name="ipool", bufs=10))
        ppool = ctx.enter_context(tc.tile_pool(name="ppool", bufs=6))
        psw = ctx.enter_context(tc.tile_pool(name="psw", bufs=3, space="PSUM"))
        psm = ctx.enter_context(tc.tile_pool(name="psm", bufs=2, space="PSUM"))
        pst = ctx.enter_context(tc.tile_pool(name="pst", bufs=2, space="PSUM"))
        psl = ctx.enter_context(tc.tile_pool(name="psl", bufs=1, space="PSUM"))
        outp = ctx.enter_context(tc.tile_pool(name="outp", bufs=4))

        # tile prefix per (s, w); call chunks broken at quarter boundaries
        pw = []
        for s in range(NSEC):
            p = [0]
            for w in range(g.nw):
                p.append(p[-1] + T[s][w])
            pw.append(p)
        call_q = [[] for _ in range(NSEC)]    # [s][q] -> list of (t0, nt)
        for s in range(NSEC):
            for q in range(NSEC):
                wlo = q * g.wpq
                whi = min((q + 1) * g.wpq, g.nw)
                t0, t1 = pw[s][wlo], pw[s][whi]
                qc = []
                t = t0
                while t < t1:
                    nt = min(TILES_PER_CALL, t1 - t)
                    qc.append((t, nt))
                    t += nt
                call_q[s].append(qc)
        # first section with tiles, per window
        fsec = [None] * g.nw
        for w in range(g.nw):
            for s in range(NSEC):
                if T[s][w] > 0:
                    fsec[w] = s
                    break

        def post(li, w, emit_ag):
            # out = wc^T @ acc_w; + bias (+relu)
            if fsec[w] is None:
                nc.vector.memset(acc[:, w * 128 : (w + 1) * 128], 0.0)
            po = psm.tile([128, 128], dt.float32, tag="po")
            nc.tensor.matmul(
                po[:],
                lhsT=wc_sb[li][:],
                rhs=acc[:, w * 128 : (w + 1) * 128],
                start=True,
                stop=True,
            )
            sl128 = slice(w * 128, (w + 1) * 128)
            if li == 0:
                rb = outp.tile([128, 128], dt.bfloat16, tag="rb")
                nc.vector.tensor_scalar(
                    rb[:],
                    po[:],
                    bc_sb[0][:],
                    0.0,
                    mybir.AluOpType.add,
                    mybir.AluOpType.max,
                )
                tp = pst.tile([128, 128], dt.bfloat16, tag="tp")
                nc.tensor.transpose(tp[:], rb[:], ident_sb[:])
                rt = outp.tile([128, 128], dt.bfloat16, tag="rt")
                nc.vector.tensor_copy(rt[:], tp[:])
                q = min(w // g.wpq, NSEC - 1)
                lw = w - q * g.wpq
                nc.sync.dma_start(
                    r2shard[q][lw * 128 : (lw + 1) * 128, :], rt[:]
                )
            else:
                nc.vector.tensor_scalar(
                    z_sb[:, sl128],
                    po[:],
                    bc_sb[1][:],
                    None,
                    mybir.AluOpType.add,
                )
                if w == g.nw - 1:
                    nc.vector.tensor_tensor(
                        z_sb[:, sl128],
                        z_sb[:, sl128],
                        lastmask_sb[:],
                        op=mybir.AluOpType.mult,
                    )
            if emit_ag:
                if NAG == NSEC:
                    for j in range(NSEC):
                        if w + 1 == min((j + 1) * g.wpq, g.nw):
                            nc.gpsimd.collective_compute(
                                "AllGather",
                                mybir.AluOpType.bypass,
                                replica_groups=rg,
                                ins=[r2shard[j][:].opt()],
                                outs=[r2full[j][:].opt()],
                            )
                elif w == g.nw - 1:
                    nc.gpsimd.collective_compute(
                        "AllGather",
                        mybir.AluOpType.bypass,
                        replica_groups=rg,
                        ins=[r2shard_1[:].opt()],
                        outs=[r2full_1[:].opt()],
                    )

        def layer(li, src_of, emit_ag):
            gts = {}

            def emit_gathers(s, q):
                for (t0, nt) in call_q[s][q]:
                    it = ipool.tile([128, TILES_PER_CALL * 8], dt.int16, tag="it")
                    nc.sync.dma_start(
                        it[:, : nt * 8],
                        idx_d[:, (tbase[s] + t0) * 8 : (tbase[s] + t0 + nt) * 8],
                    )
                    gt = gpool.tile(
                        [128, TILES_PER_CALL, DF], dt.bfloat16, tag="gt"
                    )
                    nc.gpsimd.dma_gather(
                        gt[:, :nt, :],
                        src_of(s),
                        it[:, : nt * 8],
                        nt * 128,
                        nt * 128,
                        DF,
                        single_packet=False,
                    )
                    gts[(s, t0)] = gt

            def sweep(s, q, do_post):
                wlo = q * g.wpq
                whi = min((q + 1) * g.wpq, g.nw)
                qt0 = pw[s][wlo]
                for w in range(wlo, whi):
                    tw = T[s][w]
                    if tw > 0:
                        ps = psw.tile([128, 128], dt.float32, tag="ps")
                        for t in range(tw):
                            c = pw[s][w] + t
                            cstart = qt0 + ((c - qt0) // TILES_PER_CALL) * TILES_PER_CALL
                            sl = c - cstart
                            col = tbase[s] + c
                            P = ppool.tile([128, 128], dt.bfloat16, tag="P")
                            nc.vector.tensor_scalar(
                                P[:],
                                iota_sb[:],
                                dl_sb[:, col : col + 1],
                                wv_sb[:, col : col + 1],
                                mybir.AluOpType.is_equal,
                                mybir.AluOpType.mult,
                            )
                            nc.tensor.matmul(
                                ps[:],
                                lhsT=gts[(s, cstart)][:, sl, :],
                                rhs=P[:],
                                start=(t == 0),
                                stop=(t == tw - 1),
                            )
                        sl128 = slice(w * 128, (w + 1) * 128)
                        if fsec[w] == s:
                            nc.vector.tensor_copy(acc[:, sl128], ps[:])
                        else:
                            nc.vector.tensor_tensor(
                                acc[:, sl128],
                                acc[:, sl128],
                                ps[:],
                                op=mybir.AluOpType.add,
                            )
                    if do_post:
                        post(li, w, emit_ag)

            # group order: L1 quarter-major (early AGs), L2 section-major
            if li == 0:
                groups = [(q, s) for q in range(NSEC) for s in range(NSEC)]
                order = [(s, q) for (q, s) in groups]
            else:
                order = [(s, q) for s in range(NSEC) for q in range(NSEC)]
            emit_gathers(*order[0])
            for i, (s, q) in enumerate(order):
                if i + 1 < len(order):
                    emit_gathers(*order[i + 1])
                sweep(s, q, do_post=(s == NSEC - 1))

        layer(0, lambda s: x2[s * g.sec : (s + 1) * g.sec, :], emit_ag=True)
        layer(1, lambda s: r2full[s][:], emit_ag=False)

        # ---- DGI readout ----
        fin = ctx.enter_context(tc.tile_pool(name="fin", bufs=1))
        cs = fin.tile([128, 1], dt.float32, tag="cs")
        nc.vector.reduce_sum(cs[:], z_sb[:], axis=mybir.AxisListType.X)
        nc.sync.dma_start(cs_in[:], cs[:])
        nc.gpsimd.collective_compute(
            "AllReduce",
            mybir.AluOpType.add,
            replica_groups=rg,
            ins=[cs_in[:].opt()],
            outs=[cs_out[:].opt()],
        )
        cst = fin.tile([128, 1], dt.float32, tag="cst")
        nc.sync.dma_start(cst[:], cs_out[:])
        summ = fin.tile([128, 1], dt.float32, tag="summ")
        nc.scalar.activation(
            summ[:], cst[:], mybir.ActivationFunctionType.Sigmoid, scale=inv_n
        )
        wsps = psl.tile([DF, 1], dt.float32, tag="pls")
        nc.tensor.matmul(
            wsps[:], lhsT=wstack_sb[:], rhs=summ[0:D, 0:1], start=True, stop=True
        )
        ws2 = fin.tile([DF, 2], dt.float32, tag="ws2")
        nc.vector.tensor_tensor(
            ws2[:],
            colmask_sb[:],
            wsps[:].to_broadcast([DF, 2]),
            op=mybir.AluOpType.mult,
        )
        tp_sb = fin.tile([128, g.nw], dt.float32, tag="tp_sb")
        tn_sb = fin.tile([128, g.nw], dt.float32, tag="tn_sb")
        for dti in range(g.nw):
            sl = slice(dti * 128, (dti + 1) * 128)
            tps = psl.tile([128, 2], dt.float32, tag="pls")
            nc.tensor.matmul(
                tps[:], lhsT=z_sb[:, sl], rhs=ws2[:], start=True, stop=True
            )
            nc.vector.tensor_copy(tp_sb[:, dti : dti + 1], tps[:, 0:1])
            nc.vector.tensor_copy(tn_sb[:, dti : dti + 1], tps[:, 1:2])

        # softplus(sgn*t) = relu(sgn*t) + ln1p(exp(-|t|)); deg-7 poly for ln1p
        LN1P = [
            5.62195900721818e-07, 0.9999574870750696, -0.4992065685478763,
            0.32697310001391783, -0.2228362583278401, 0.13076503250360005,
            -0.05262485136716543, 0.010119082927575069,
        ]

        def softplus_of(t_in, sgn, tagp):
            neg = fin.tile([128, g.nw], dt.float32, tag=f"{tagp}neg")
            nc.vector.tensor_scalar(
                neg[:], t_in[:], -1.0, None, mybir.AluOpType.mult
            )
            ab = fin.tile([128, g.nw], dt.float32, tag=f"{tagp}ab")
            nc.vector.tensor_tensor(ab[:], t_in[:], neg[:], op=mybir.AluOpType.max)
            uu = fin.tile([128, g.nw], dt.float32, tag=f"{tagp}uu")
            nc.scalar.activation(
                uu[:], ab[:], mybir.ActivationFunctionType.Exp, scale=-1.0
            )
            pp_ = fin.tile([128, g.nw], dt.float32, tag=f"{tagp}pp")
            nc.vector.tensor_scalar(
                pp_[:], uu[:], LN1P[7], LN1P[6],
                mybir.AluOpType.mult, mybir.AluOpType.add,
            )
            pm = fin.tile([128, g.nw], dt.float32, tag=f"{tagp}pm")
            for ci in range(5, -1, -1):
                nc.vector.tensor_tensor(
                    pm[:], pp_[:], uu[:], op=mybir.AluOpType.mult
                )
                nc.vector.tensor_scalar(
                    pp_[:], pm[:], LN1P[ci], None, mybir.AluOpType.add
                )
            rl = fin.tile([128, g.nw], dt.float32, tag=f"{tagp}rl")
            nc.vector.tensor_scalar(
                rl[:], (t_in if sgn > 0 else neg)[:], 0.0, None,
                mybir.AluOpType.max,
            )
            res = fin.tile([128, g.nw], dt.float32, tag=f"{tagp}res")
            nc.vector.tensor_tensor(res[:], rl[:], pp_[:], op=mybir.AluOpType.add)
            return res

        spp = softplus_of(tp_sb, -1, "sp")   # softplus(-t_pos)
        spn = softplus_of(tn_sb, +1, "sn")   # softplus(t_neg)
        ssum = fin.tile([128, g.nw], dt.float32, tag="ssum")
        nc.vector.tensor_tensor(ssum[:], spp[:], spn[:], op=mybir.AluOpType.add)
        nc.vector.tensor_tensor(
            ssum[:], ssum[:], mask_sb[:], op=mybir.AluOpType.mult
        )
        srow = fin.tile([128, 1], dt.float32, tag="srow")
        nc.vector.reduce_sum(srow[:], ssum[:], axis=mybir.AxisListType.X)
        tot = psl.tile([1, 1], dt.float32, tag="pls")
        nc.tensor.matmul(
            tot[:], lhsT=srow[:], rhs=ones_sb[:], start=True, stop=True
        )
        lsb = fin.tile([1, 16], dt.float32, tag="lsb")
        nc.vector.memset(lsb[:], 0.0)
        nc.vector.tensor_copy(lsb[0:1, 0:1], tot[:])
        nc.sync.dma_start(ls_in[:], lsb[:])
        nc.gpsimd.collective_compute(
            "AllReduce",
            mybir.AluOpType.add,
            replica_groups=rg,
            ins=[ls_in[:].opt()],
            outs=[ls_out[:].opt()],
        )
        lsf = fin.tile([1, 16], dt.float32, tag="lsf")
        nc.sync.dma_start(lsf[:], ls_out[:])
        lout = fin.tile([1, 16], dt.float32, tag="lout")
        nc.scalar.activation(
            lout[:], lsf[:], mybir.ActivationFunctionType.Copy, scale=inv_n
        )
        nc.sync.dma_start(loss_out, lout[:])

    nc.compile()
    return nc


_prog_cache = {}


def _get_prog(g, struct):
    key = (g.npc, g.nreal, struct)
    if key not in _prog_cache:
        _prog_cache[key] = _build(g, struct)
    return _prog_cache[key]


def run(inputs, npc, nreal, trace=False):
    g = Geo(npc, nreal)
    in_maps, struct = _preprocess(g, **inputs)
    nc = _get_prog(g, struct)
    res = run_bass_kernel_spmd(
        nc, in_maps, core_ids=list(range(C)), trace=trace
    )
    loss = res.results[0]["loss"][0, 0]
    return np.float32(loss), res


def kernel(**inputs):
    out, _ = run(inputs, npc=12500, nreal=100000)
    return out


def _make_sharded_exec(nc, in_maps, reps=1):
    """Reusable jitted shard_map executor mirroring bass2jax's multi-core
    path, with device-resident inputs."""
    import jax
    from jax.experimental.shard_map import shard_map
    from jax.sharding import Mesh, NamedSharding, PartitionSpec

    from concourse import bass2jax, mybir as _mb

    bass2jax.install_neuronx_cc_hook()
    partition_name = (
        nc.partition_id_tensor.name if nc.partition_id_tensor else None
    )
    in_names, out_names, out_avals, zero_shapes = [], [], [], []
    for alloc in nc.m.functions[0].allocations:
        if not isinstance(alloc, _mb.MemoryLocationSet):
            continue
        name = alloc.memorylocations[0].name
        if alloc.kind == "ExternalInput":
            if name != partition_name:
                in_names.append(name)
        elif alloc.kind == "ExternalOutput":
            shape = tuple(alloc.tensor_shape)
            dty = _mb.dt.np(alloc.dtype)
            out_names.append(name)
            out_avals.append(jax.core.ShapedArray(shape, dty))
            zero_shapes.append((shape, dty))
    n_params = len(in_names)
    n_outs = len(out_avals)
    all_names = list(in_names) + list(out_names)
    if partition_name is not None:
        all_names.append(partition_name)
    donate = tuple(range(n_params, n_params + n_outs * reps))

    assert reps == 1  # the neuronx_cc hook allows one bass_exec per module

    def _body(*args):
        operands = list(args)
        if partition_name is not None:
            operands.append(bass2jax.partition_id_tensor())
        outs = bass2jax._bass_exec_p.bind(
            *operands,
            out_avals=tuple(out_avals),
            in_names=tuple(all_names),
            out_names=tuple(out_names),
            lowering_input_output_aliases=(),
            sim_require_finite=True,
            sim_require_nnan=True,
            nc=nc,
        )
        return tuple(outs)

    devices = jax.devices()[:C]
    mesh = Mesh(np.array(devices), ("core",))
    spec = PartitionSpec("core")
    sharded = jax.jit(
        shard_map(
            _body,
            mesh=mesh,
            in_specs=(spec,) * (n_params + n_outs * reps),
            out_specs=(spec,) * n_outs,
            check_rep=False,
        ),
        donate_argnums=donate,
        keep_unused=True,
    )
    shard = NamedSharding(mesh, spec)
    concat_in = [
        jax.device_put(
            np.concatenate([np.asarray(m[nm]) for m in in_maps], axis=0), shard
        )
        for nm in in_names
    ]

    def launch():
        zeros = [
            jax.device_put(np.zeros((C * s[0], *s[1:]), d), shard)
            for (s, d) in zero_shapes
        ]
        return sharded(*concat_in, *zeros)

    def fetch(outs):
        jax.block_until_ready(outs)
        return {
            nm: np.asarray(outs[i]).reshape(C, *out_avals[i].shape)[0]
            for i, nm in enumerate(out_names)
        }

    def run_once():
        return fetch(launch())

    run_once.launch = launch
    run_once.fetch = fetch
    return run_once


def bench(inputs, npc=12500, nreal=100000, iters=6):
    import time

    g = Geo(npc, nreal)
    t0 = time.time()
    in_maps, struct = _preprocess(g, **inputs)
    t1 = time.time()
    nc = _get_prog(g, struct)
    t2 = time.time()
    run_1 = _make_sharded_exec(nc, in_maps)
    out = run_1()  # warmup: compiles + loads NEFF
    t3 = time.time()
    t1s = []
    for _ in range(iters):
        ta = time.time()
        out = run_1()
        t1s.append(time.time() - ta)
    K = 48
    ta = time.time()
    pend = [run_1.launch() for _ in range(K)]
    import jax as _jax
    _jax.block_until_ready(pend)
    tK = time.time() - ta
    per = (tK - min(t1s)) / (K - 1)
    print(
        f"preprocess {t1-t0:.1f}s  build {t2-t1:.1f}s  warmup {t3-t2:.1f}s\n"
        f"  1-shot ms: {[round(t*1e3,2) for t in t1s]}\n"
        f"  {K} pipelined: total {tK*1e3:.1f} ms -> marginal {per*1e3:.3f} ms"
    )
    return np.float32(out["loss"][0, 0]), per


# revision 25
# speedup vs baseline: 1.9393x; 1.0426x over previous
"""DeepGraphInfomax loss (2-layer GCN encoder, pos+neg, DGI readout) on 8 trn2 cores.

Window-major dst-sharded pull-mode GNN aggregation:
  - Nodes (dst rows) sharded contiguously across 8 cores (12500 each).
  - pos/neg feature streams fused into 128-wide rows: X2[r] = [x[r] | x[perm[r]]].
  - Self-loops folded in as explicit edges with degree product deg^2, so the
    aggregation produces the complete GCN pre-activation in one pass.
  - Source rows live in a quarter-major layout: node (core k, local l) maps to
    row 25600*(l//3200) + 3200*k + (l%3200).  The 4 sections of 25600 rows keep
    int16 gather indices valid, AND layer-1 (x2q) and layer-2 (r2full) share
    the exact same index space, so idx/dstl/norm arrays are staged and loaded
    once for both layers.
  - Processing is window-major: all tiles of one 128-dst window (across all 4
    source sections) accumulate into a single PSUM tile via one-hot matmuls
    with swapped operands (lhsT=gathered rows, rhs=one-hot), yielding
    feature-major results directly.  No DRAM accumulator, no scatter-add.
  - post per window: PE applies W (A @ (X W) == (A @ X) W) straight from the
    SBUF accumulator, DVE applies bias(+relu); layer-1 results are transposed
    on the PE (not the DMA xbar: Tile serializes DMA-transposes with in-flight
    collectives) and stored row-major bf16 to r2shard.
  - r2shard is AllGathered in 4 quarter chunks, each gated only on the quarter
    of post-L1 windows it needs, so layer-2 gathers start while layer-1 post
    is still finishing.
  - DGI readout (summary / W_dgi / softplus losses) computed on device with two
    tiny AllReduces.

Host-side preprocessing only manipulates integer graph structure (sorting,
degree counts, packing, index mapping) and stages dtype-cast copies of the
inputs; all floating-point math of the reference runs on device.
"""

import sys

for _p in ("/opt/trn_rl_repo", "/root/.axon_site/_ro/trn_rl_repo"):
    if _p not in sys.path:
        sys.path.insert(0, _p)

from contextlib import ExitStack

import ml_dtypes
import numpy as np

import concourse.bass as bass
import concourse.bacc as bacc
import concourse.mybir as mybir
import concourse.tile as tile
from concourse.bass_utils import run_bass_kernel_spmd

BF16 = ml_dtypes.bfloat16
F32 = np.float32

C = 8            # cores
D = 64           # hidden dim
DF = 2 * D       # fused pos|neg width
NSEC = 4
NAG = 4          # AllGather chunks (1 or NSEC)
TILES_PER_CALL = 48
SLOTS_PER_CALL = TILES_PER_CALL * 128
PAD_DEG = 1e30   # pad-slot degree product -> norm ~ 1e-15 ~ 0


class Geo:
    def __init__(self, npc, nreal):
        self.npc = npc                       # real nodes per core
        self.nreal = nreal                   # total real nodes (= 8*npc)
        self.nw = -(-npc // 128)             # dst windows per core (98)
        self.ldim = 128 * self.nw            # padded dsts per core (12544)
        self.wpq = -(-self.nw // NSEC)       # windows per quarter (25)
        self.ql = self.wpq * 128             # locals per quarter (3200)
        self.sec = C * self.ql               # rows per section (25600)
        self.xrows = NSEC * self.sec         # padded source-row space (102400)
        self.shard = NSEC * self.ql          # r2shard rows (12800)
        assert self.sec < 32768


def _preprocess(g, x, W1, b1, W2, b2, W_dgi, edge_index, perm):
    """Build per-core device inputs. Integer index work + dtype staging only."""
    row = np.asarray(edge_index[0], dtype=np.int64)
    col = np.asarray(edge_index[1], dtype=np.int64)
    perm = np.asarray(perm, dtype=np.int64)
    N = g.nreal
    npc, ql = g.npc, g.ql

    deg = np.bincount(col, minlength=N).astype(np.int64) + 1  # in-deg + 1

    # source-row id per global node: quarter-major for NAG=4 (chunked
    # AllGathers concat per-quarter), core-major for NAG=1 (single AllGather
    # concatenates full shards)
    gids = np.arange(N, dtype=np.int64)
    kk = gids // npc
    ll = gids % npc
    if NAG == 1:
        r2p = g.shard * kk + ll
    else:
        r2p = g.sec * (ll // ql) + ql * kk + (ll % ql)

    # fused bf16 feature rows in quarter-major layout
    X2 = np.zeros((g.xrows, DF), dtype=BF16)
    X2[r2p, :D] = x.astype(BF16)
    X2[r2p, D:] = x[perm].astype(BF16)

    # edges + self-loops (self: src == dst, degp = deg^2 -> weight 1/deg)
    rows_a = np.concatenate([row, gids])
    cols_a = np.concatenate([col, gids])
    src_q = r2p[rows_a]                       # quarter-major src row
    kd = cols_a // npc                        # dst core
    dl = cols_a % npc                         # dst local
    sec = src_q // g.sec
    w = dl // 128

    # tile counts per (core, sec, window) -> T = max over cores
    key = ((kd * NSEC + sec) * g.nw + w).astype(np.int64)
    cnt = np.bincount(key, minlength=C * NSEC * g.nw).reshape(C, NSEC, g.nw)
    T = np.maximum(-(-cnt // 128), 0).max(axis=0)           # [NSEC, NW]
    tiles_s = T.sum(axis=1)                                 # tiles per section
    ntiles = int(tiles_s.sum())
    calls = [
        [
            TILES_PER_CALL
            if (c + 1) * TILES_PER_CALL <= tiles_s[s]
            else int(tiles_s[s] - c * TILES_PER_CALL)
            for c in range(-(-int(tiles_s[s]) // TILES_PER_CALL))
        ]
        for s in range(NSEC)
    ]
    tbase = np.concatenate([[0], np.cumsum(tiles_s)])       # section tile base
    # slot base of each (s, w) run
    wbase = np.zeros((NSEC, g.nw), dtype=np.int64)
    for s in range(NSEC):
        wbase[s] = (tbase[s] + np.concatenate([[0], np.cumsum(T[s])[:-1]])) * 128

    deg_f = deg.astype(np.float64)
    degp_a = deg_f[rows_a] * deg_f[cols_a]

    ins = []
    for k in range(C):
        m = kd == k
        sq, dk, wk, sk = src_q[m], dl[m], w[m], sec[m]
        dp = degp_a[m]
        order = np.lexsort((sq, dk, wk, sk))
        sq, dk, wk, sk, dp = (a[order] for a in (sq, dk, wk, sk, dp))
        # rank within each (sec, window) run
        runkey = sk * g.nw + wk
        starts = np.searchsorted(runkey, runkey, side="left")
        rank = np.arange(len(runkey)) - starts
        slot = wbase[sk, wk] + rank

        S = ntiles * 128
        idx = np.zeros(S, dtype=np.int16)
        dstl = np.zeros(S, dtype=np.int32)
        degp = np.full(S, PAD_DEG, dtype=F32)
        idx[slot] = (sq - sk * g.sec).astype(np.int16)
        dstl[slot] = dk - wk * 128
        degp[slot] = dp.astype(F32)
        assert dstl.min() >= 0 and dstl.max() < 128

        d_in = {
            # wrapped int16 index layout: slot j -> [j%16, j//16], replicated x8
            "idx": np.ascontiguousarray(
                np.tile(idx.reshape(-1, 16).T, (8, 1)).astype(np.int16)
            ),
            "dstl": np.ascontiguousarray(dstl.reshape(-1, 128).T.astype(F32)),
            "degp": np.ascontiguousarray(degp.reshape(-1, 128).T),
        }
        ins.append(d_in)

    # shared constants
    iota = np.tile(np.arange(128, dtype=F32), (128, 1)).astype(BF16)
    ident = np.eye(128, dtype=F32).astype(BF16)
    wc1 = np.zeros((DF, DF), dtype=F32)
    wc1[:D, :D] = W1
    wc1[D:, D:] = W1
    wc2 = np.zeros((DF, DF), dtype=F32)
    wc2[:D, :D] = W2
    wc2[D:, D:] = W2
    bc1 = np.concatenate([b1, b1]).astype(F32).reshape(DF, 1)
    bc2 = np.concatenate([b2, b2]).astype(F32).reshape(DF, 1)
    wstack = np.zeros((D, DF), dtype=F32)
    wstack[:, :D] = W_dgi.T
    wstack[:, D:] = W_dgi.T
    colmask = np.zeros((DF, 2), dtype=F32)
    colmask[:D, 0] = 1.0
    colmask[D:, 1] = 1.0
    nvalid_last = g.npc - (g.nw - 1) * 128
    lastmask = np.tile((np.arange(128) < nvalid_last).astype(F32), (128, 1))
    mk = (np.arange(g.ldim) < g.npc).astype(F32)
    shared = {
        "x2": X2,
        "iota": iota,
        "ident": ident,
        "wc1": wc1,
        "wc2": wc2,
        "bc1": bc1,
        "bc2": bc2,
        "wstack": wstack,
        "colmask": colmask,
        "lastmask": lastmask,
        "mask": np.ascontiguousarray(mk.reshape(g.nw, 128).T),
        "ones": np.ones((128, 1), dtype=F32),
    }
    for d_in in ins:
        d_in.update(shared)
    struct = (tuple(map(tuple, T)), tuple(map(tuple, calls)))
    return ins, struct


def _build(g, struct):
    T, calls = struct
    T = [list(r) for r in T]
    calls = [list(r) for r in calls]
    tiles_s = [sum(r) for r in T]
    ntiles = sum(tiles_s)
    tbase = [0]
    for s in range(NSEC):
        tbase.append(tbase[-1] + tiles_s[s])

    dt = mybir.dt
    nc = bacc.Bacc(
        "TRN2", target_bir_lowering=False, debug=False, num_devices=C
    )

    def din(name, shape, dty):
        return nc.dram_tensor(name, list(shape), dty, kind="ExternalInput").ap()

    x2 = din("x2", (g.xrows, DF), dt.bfloat16)
    idx_d = din("idx", (128, ntiles * 8), dt.int16)
    dstl_d = din("dstl", (128, ntiles), dt.float32)
    degp_d = din("degp", (128, ntiles), dt.float32)
    iota_d = din("iota", (128, 128), dt.bfloat16)
    ident_d = din("ident", (128, 128), dt.bfloat16)
    wc_d = [din("wc1", (DF, DF), dt.float32), din("wc2", (DF, DF), dt.float32)]
    bc_d = [din("bc1", (DF, 1), dt.float32), din("bc2", (DF, 1), dt.float32)]
    wstack_d = din("wstack", (D, DF), dt.float32)
    colmask_d = din("colmask", (DF, 2), dt.float32)
    lastmask_d = din("lastmask", (128, 128), dt.float32)
    mask_d = din("mask", (128, g.nw), dt.float32)
    ones_d = din("ones", (128, 1), dt.float32)
    loss_out = nc.dram_tensor("loss", [1, 16], dt.float32, kind="ExternalOutput").ap()

    inv_n = 1.0 / float(g.nreal)
    rg = [list(range(C))]

    with tile.TileContext(nc) as tc, ExitStack() as ctx:
        dram = ctx.enter_context(tc.tile_pool(name="dram", bufs=1, space="DRAM"))
        if NAG == NSEC:
            r2shard = []
            for j in range(NSEC):
                r2s_j = dram.tile(
                    [g.ql, DF], dt.bfloat16, tag=f"r2shard{j}",
                    name=f"r2shard_{j}",
                )
                r2shard.append(r2s_j)
            r2full = []
            for j in range(NSEC):
                r2f_j = dram.tile(
                    [g.sec, DF], dt.bfloat16, tag=f"r2full{j}",
                    addr_space="Shared", name=f"r2full_{j}",
                )
                r2full.append(r2f_j)
        else:
            r2shard_1 = dram.tile(
                [g.shard, DF], dt.bfloat16, tag="r2shard", name="r2shard_1"
            )
            r2full_1 = dram.tile(
                [g.xrows, DF], dt.bfloat16, tag="r2full",
                addr_space="Shared", name="r2full_1",
            )
            r2shard = [
                r2shard_1[j * g.ql : (j + 1) * g.ql, :] for j in range(NSEC)
            ]
            r2full = [
                r2full_1[j * g.sec : (j + 1) * g.sec, :] for j in range(NSEC)
            ]
        cs_in = dram.tile([128, 1], dt.float32, tag="cs_in")
        cs_out = dram.tile([128, 1], dt.float32, tag="cs_out", addr_space="Shared")
        ls_in = dram.tile([1, 16], dt.float32, tag="ls_in")
        ls_out = dram.tile([1, 16], dt.float32, tag="ls_out", addr_space="Shared")

        const = ctx.enter_context(tc.tile_pool(name="const", bufs=1))

        def cload(ap_dram, shape, dty, tag):
            t = const.tile(list(shape), dty, tag=tag)
            nc.sync.dma_start(t[:], ap_dram)
            return t

        iota_sb = cload(iota_d, (128, 128), dt.bfloat16, "iota")
        ident_sb = cload(ident_d, (128, 128), dt.bfloat16, "ident")
        wc_sb = [
            cload(wc_d[0], (DF, DF), dt.float32, "wc1"),
            cload(wc_d[1], (DF, DF), dt.float32, "wc2"),
        ]
        bc_sb = [
            cload(bc_d[0], (DF, 1), dt.float32, "bc1"),
            cload(bc_d[1], (DF, 1), dt.float32, "bc2"),
        ]
        wstack_sb = cload(wstack_d, (D, DF), dt.float32, "wstack")
        colmask_sb = cload(colmask_d, (DF, 2), dt.float32, "colmask")
        lastmask_sb = cload(lastmask_d, (128, 128), dt.float32, "lastmask")
        mask_sb = cload(mask_d, (128, g.nw), dt.float32, "mask")
        ones_sb = cload(ones_d, (128, 1), dt.float32, "ones")

        big = ctx.enter_context(tc.tile_pool(name="big", bufs=1))
        z_sb = big.tile([128, g.ldim], dt.float32, tag="z_sb")
        acc = big.tile([128, g.ldim], dt.float32, tag="acc")
        dl_sb = big.tile([128, ntiles], dt.float32, tag="dl_sb")
        nc.sync.dma_start(dl_sb[:], dstl_d)
        wv_sb = big.tile([128, ntiles], dt.float32, tag="wv_sb")
        nc.sync.dma_start(wv_sb[:], degp_d)
        nc.vector.reciprocal(wv_sb[:], wv_sb[:])
        nc.scalar.sqrt(wv_sb[:], wv_sb[:])

        gpool = ctx.enter_context(tc.tile_pool(name="gpool", bufs=6))
        ipool = ctx.enter_context(tc.tile_pool(name="ipool", bufs=10))
        ppool = ctx.enter_context(tc.tile_pool(name="ppool", bufs=6))
        psw = ctx.enter_context(tc.tile_pool(name="psw", bufs=4, space="PSUM"))
        psm = ctx.enter_context(tc.tile_pool(name="psm", bufs=2, space="PSUM"))
        pst = ctx.enter_context(tc.tile_pool(name="pst", bufs=1, space="PSUM"))
        psl = ctx.enter_context(tc.tile_pool(name="psl", bufs=1, space="PSUM"))
        outp = ctx.enter_context(tc.tile_pool(name="outp", bufs=4))

        # tile prefix per (s, w); call chunks broken at quarter boundaries
        pw = []
        for s in range(NSEC):
            p = [0]
            for w in range(g.nw):
                p.append(p[-1] + T[s][w])
            pw.append(p)
        call_q = [[] for _ in range(NSEC)]    # [s][q] -> list of (t0, nt)
        for s in range(NSEC):
            for q in range(NSEC):
                wlo = q * g.wpq
                whi = min((q + 1) * g.wpq, g.nw)
                t0, t1 = pw[s][wlo], pw[s][whi]
                qc = []
                t = t0
                while t < t1:
                    nt = min(TILES_PER_CALL, t1 - t)
                    qc.append((t, nt))
                    t += nt
                call_q[s].append(qc)
        # first section with tiles, per window
        fsec = [None] * g.nw
        for w in range(g.nw):
            for s in range(NSEC):
                if T[s][w] > 0:
                    fsec[w] = s
                    break

        def post(li, w, emit_ag):
            # out = wc^T @ acc_w; + bias (+relu)
            if fsec[w] is None:
                nc.vector.memset(acc[:, w * 128 : (w + 1) * 128], 0.0)
            po = psm.tile([128, 128], dt.float32, tag="po")
            nc.tensor.matmul(
                po[:],
                lhsT=wc_sb[li][:],
                rhs=acc[:, w * 128 : (w + 1) * 128],
                start=True,
                stop=True,
            )
            sl128 = slice(w * 128, (w + 1) * 128)
            if li == 0:
                rb = outp.tile([128, 128], dt.bfloat16, tag="rb")
                nc.scalar.activation(
                    rb[:],
                    po[:],
                    mybir.ActivationFunctionType.Relu,
                    bias=bc_sb[0][:],
                )
                tp = pst.tile([128, 128], dt.bfloat16, tag="tp")
                nc.tensor.transpose(tp[:], rb[:], ident_sb[:])
                rt = outp.tile([128, 128], dt.bfloat16, tag="rt")
                nc.scalar.activation(
                    rt[:], tp[:], mybir.ActivationFunctionType.Copy
                )
                q = min(w // g.wpq, NSEC - 1)
                lw = w - q * g.wpq
                nc.sync.dma_start(
                    r2shard[q][lw * 128 : (lw + 1) * 128, :], rt[:]
                )
            else:
                nc.vector.tensor_scalar(
                    z_sb[:, sl128],
                    po[:],
                    bc_sb[1][:],
                    None,
                    mybir.AluOpType.add,
                )
                if w == g.nw - 1:
                    nc.vector.tensor_tensor(
                        z_sb[:, sl128],
                        z_sb[:, sl128],
                        lastmask_sb[:],
                        op=mybir.AluOpType.mult,
                    )
            if emit_ag:
                if NAG == NSEC:
                    for j in range(NSEC):
                        if w + 1 == min((j + 1) * g.wpq, g.nw):
                            nc.gpsimd.collective_compute(
                                "AllGather",
                                mybir.AluOpType.bypass,
                                replica_groups=rg,
                                ins=[r2shard[j][:].opt()],
                                outs=[r2full[j][:].opt()],
                            )
                elif w == g.nw - 1:
                    nc.gpsimd.collective_compute(
                        "AllGather",
                        mybir.AluOpType.bypass,
                        replica_groups=rg,
                        ins=[r2shard_1[:].opt()],
                        outs=[r2full_1[:].opt()],
                    )

        def layer(li, src_of, emit_ag):
            gts = {}

            def emit_gathers(s, q):
                for (t0, nt) in call_q[s][q]:
                    it = ipool.tile([128, TILES_PER_CALL * 8], dt.int16, tag="it")
                    nc.sync.dma_start(
                        it[:, : nt * 8],
                        idx_d[:, (tbase[s] + t0) * 8 : (tbase[s] + t0 + nt) * 8],
                    )
                    gt = gpool.tile(
                        [128, TILES_PER_CALL, DF], dt.bfloat16, tag="gt"
                    )
                    nc.gpsimd.dma_gather(
                        gt[:, :nt, :],
                        src_of(s),
                        it[:, : nt * 8],
                        nt * 128,
                        nt * 128,
                        DF,
                        single_packet=False,
                    )
                    gts[(s, t0)] = gt

            def sweep(s, q, do_post):
                wlo = q * g.wpq
                whi = min((q + 1) * g.wpq, g.nw)
                qt0 = pw[s][wlo]
                for w in range(wlo, whi):
                    tw = T[s][w]
                    if tw > 0:
                        ps = psw.tile([128, 128], dt.float32, tag="ps")
                        for t in range(tw):
                            c = pw[s][w] + t
                            cstart = qt0 + ((c - qt0) // TILES_PER_CALL) * TILES_PER_CALL
                            sl = c - cstart
                            col = tbase[s] + c
                            P = ppool.tile([128, 128], dt.bfloat16, tag="P")
                            nc.vector.tensor_scalar(
                                P[:],
                                iota_sb[:],
                                dl_sb[:, col : col + 1],
                                wv_sb[:, col : col + 1],
                                mybir.AluOpType.is_equal,
                                mybir.AluOpType.mult,
                            )
                            nc.tensor.matmul(
                                ps[:],
                                lhsT=gts[(s, cstart)][:, sl, :],
                                rhs=P[:],
                                start=(t == 0),
                                stop=(t == tw - 1),
                            )
                        sl128 = slice(w * 128, (w + 1) * 128)
                        if fsec[w] == s:
                            nc.scalar.activation(
                                acc[:, sl128],
                                ps[:],
                                mybir.ActivationFunctionType.Copy,
                            )
                        else:
                            nc.vector.tensor_tensor(
                                acc[:, sl128],
                                acc[:, sl128],
                                ps[:],
                                op=mybir.AluOpType.add,
                            )
                    if do_post:
                        post(li, w, emit_ag)

            # group order: L1 quarter-major (early AGs), L2 section-major
            if li == 0:
                groups = [(q, s) for q in range(NSEC) for s in range(NSEC)]
                order = [(s, q) for (q, s) in groups]
            else:
                order = [(s, q) for s in range(NSEC) for q in range(NSEC)]
            emit_gathers(*order[0])
            for i, (s, q) in enumerate(order):
                if i + 1 < len(order):
                    emit_gathers(*order[i + 1])
                sweep(s, q, do_post=(s == NSEC - 1))

        layer(0, lambda s: x2[s * g.sec : (s + 1) * g.sec, :], emit_ag=True)
        layer(1, lambda s: r2full[s][:], emit_ag=False)

        # ---- DGI readout ----
        fin = ctx.enter_context(tc.tile_pool(name="fin", bufs=1))
        cs = fin.tile([128, 1], dt.float32, tag="cs")
        nc.vector.reduce_sum(cs[:], z_sb[:], axis=mybir.AxisListType.X)
        nc.sync.dma_start(cs_in[:], cs[:])
        nc.gpsimd.collective_compute(
            "AllReduce",
            mybir.AluOpType.add,
            replica_groups=rg,
            ins=[cs_in[:].opt()],
            outs=[cs_out[:].opt()],
        )
        cst = fin.tile([128, 1], dt.float32, tag="cst")
        nc.sync.dma_start(cst[:], cs_out[:])
        summ = fin.tile([128, 1], dt.float32, tag="summ")
        nc.scalar.activation(
            summ[:], cst[:], mybir.ActivationFunctionType.Sigmoid, scale=inv_n
        )
        wsps = psl.tile([DF, 1], dt.float32, tag="pls")
        nc.tensor.matmul(
            wsps[:], lhsT=wstack_sb[:], rhs=summ[0:D, 0:1], start=True, stop=True
        )
        ws2 = fin.tile([DF, 2], dt.float32, tag="ws2")
        nc.vector.tensor_tensor(
            ws2[:],
            colmask_sb[:],
            wsps[:].to_broadcast([DF, 2]),
            op=mybir.AluOpType.mult,
        )
        tpn_sb = fin.tile([128, g.nw, 2], dt.float32, tag="tpn_sb")
        for dti in range(g.nw):
            sl = slice(dti * 128, (dti + 1) * 128)
            tps = psl.tile([128, 2], dt.float32, tag="pls")
            nc.tensor.matmul(
                tps[:], lhsT=z_sb[:, sl], rhs=ws2[:], start=True, stop=True
            )
            nc.vector.tensor_copy(tpn_sb[:, dti, :], tps[:])
        tp_sb = tpn_sb[:, :, 0]
        tn_sb = tpn_sb[:, :, 1]

        # softplus(sgn*t) = relu(sgn*t) + ln1p(exp(-|t|)); deg-7 poly for ln1p
        LN1P = [
            5.62195900721818e-07, 0.9999574870750696, -0.4992065685478763,
            0.32697310001391783, -0.2228362583278401, 0.13076503250360005,
            -0.05262485136716543, 0.010119082927575069,
        ]

        def softplus_of(t_in, sgn, tagp):
            neg = fin.tile([128, g.nw], dt.float32, tag=f"{tagp}neg")
            nc.vector.tensor_scalar(
                neg[:], t_in, -1.0, None, mybir.AluOpType.mult
            )
            ab = fin.tile([128, g.nw], dt.float32, tag=f"{tagp}ab")
            nc.vector.tensor_tensor(ab[:], t_in, neg[:], op=mybir.AluOpType.max)
            uu = fin.tile([128, g.nw], dt.float32, tag=f"{tagp}uu")
            nc.scalar.activation(
                uu[:], ab[:], mybir.ActivationFunctionType.Exp, scale=-1.0
            )
            pp_ = fin.tile([128, g.nw], dt.float32, tag=f"{tagp}pp")
            nc.vector.tensor_scalar(
                pp_[:], uu[:], LN1P[7], LN1P[6],
                mybir.AluOpType.mult, mybir.AluOpType.add,
            )
            pm = fin.tile([128, g.nw], dt.float32, tag=f"{tagp}pm")
            for ci in range(5, -1, -1):
                nc.vector.tensor_tensor(
                    pm[:], pp_[:], uu[:], op=mybir.AluOpType.mult
                )
                nc.vector.tensor_scalar(
                    pp_[:], pm[:], LN1P[ci], None, mybir.AluOpType.add
                )
            rl = fin.tile([128, g.nw], dt.float32, tag=f"{tagp}rl")
            nc.vector.tensor_scalar(
                rl[:], t_in if sgn > 0 else neg[:], 0.0, None,
                mybir.AluOpType.max,
            )
            res = fin.tile([128, g.nw], dt.float32, tag=f"{tagp}res")
            nc.vector.tensor_tensor(res[:], rl[:], pp_[:], op=mybir.AluOpType.add)
            return res

        spp = softplus_of(tp_sb, -1, "sp")   # softplus(-t_pos)
        spn = softplus_of(tn_sb, +1, "sn")   # softplus(t_neg)
        ssum = fin.tile([128, g.nw], dt.float32, tag="ssum")
        nc.vector.tensor_tensor(ssum[:], spp[:], spn[:], op=mybir.AluOpType.add)
        nc.vector.tensor_tensor(
            ssum[:], ssum[:], mask_sb[:], op=mybir.AluOpType.mult
        )
        srow = fin.tile([128, 1], dt.float32, tag="srow")
        nc.vector.reduce_sum(srow[:], ssum[:], axis=mybir.AxisListType.X)
        tot = psl.tile([1, 1], dt.float32, tag="pls")
        nc.tensor.matmul(
            tot[:], lhsT=srow[:], rhs=ones_sb[:], start=True, stop=True
        )
        lsb = fin.tile([1, 16], dt.float32, tag="lsb")
        nc.vector.memset(lsb[:], 0.0)
        nc.vector.tensor_copy(lsb[0:1, 0:1], tot[:])
        nc.sync.dma_start(ls_in[:], lsb[:])
        nc.gpsimd.collective_compute(
            "AllReduce",
            mybir.AluOpType.add,
            replica_groups=rg,
            ins=[ls_in[:].opt()],
            outs=[ls_out[:].opt()],
        )
        lsf = fin.tile([1, 16], dt.float32, tag="lsf")
        nc.sync.dma_start(lsf[:], ls_out[:])
        lout = fin.tile([1, 16], dt.float32, tag="lout")
        nc.scalar.activation(
            lout[:], lsf[:], mybir.ActivationFunctionType.Copy, scale=inv_n
        )
        nc.sync.dma_start(loss_out, lout[:])

    nc.compile()
    return nc


_prog_cache = {}


def _get_prog(g, struct):
    key = (g.npc, g.nreal, struct)
    if key not in _prog_cache:
        _prog_cache[key] = _build(g, struct)
    return _prog_cache[key]


def run(inputs, npc, nreal, trace=False):
    g = Geo(npc, nreal)
    in_maps, struct = _preprocess(g, **inputs)
    nc = _get_prog(g, struct)
    res = run_bass_kernel_spmd(
        nc, in_maps, core_ids=list(range(C)), trace=trace
    )
    loss = res.results[0]["loss"][0, 0]
    return np.float32(loss), res


def kernel(**inputs):
    out, _ = run(inputs, npc=12500, nreal=100000)
    return out


def _make_sharded_exec(nc, in_maps, reps=1):
    """Reusable jitted shard_map executor mirroring bass2jax's multi-core
    path, with device-resident inputs."""
    import jax
    from jax.experimental.shard_map import shard_map
    from jax.sharding import Mesh, NamedSharding, PartitionSpec

    from concourse import bass2jax, mybir as _mb

    bass2jax.install_neuronx_cc_hook()
    partition_name = (
        nc.partition_id_tensor.name if nc.partition_id_tensor else None
    )
    in_names, out_names, out_avals, zero_shapes = [], [], [], []
    for alloc in nc.m.functions[0].allocations:
        if not isinstance(alloc, _mb.MemoryLocationSet):
            continue
        name = alloc.memorylocations[0].name
        if alloc.kind == "ExternalInput":
            if name != partition_name:
                in_names.append(name)
        elif alloc.kind == "ExternalOutput":
            shape = tuple(alloc.tensor_shape)
            dty = _mb.dt.np(alloc.dtype)
            out_names.append(name)
            out_avals.append(jax.core.ShapedArray(shape, dty))
            zero_shapes.append((shape, dty))
    n_params = len(in_names)
    n_outs = len(out_avals)
    all_names = list(in_names) + list(out_names)
    if partition_name is not None:
        all_names.append(partition_name)
    donate = tuple(range(n_params, n_params + n_outs * reps))

    assert reps == 1  # the neuronx_cc hook allows one bass_exec per module

    def _body(*args):
        operands = list(args)
        if partition_name is not None:
            operands.append(bass2jax.partition_id_tensor())
        outs = bass2jax._bass_exec_p.bind(
            *operands,
            out_avals=tuple(out_avals),
            in_names=tuple(all_names),
            out_names=tuple(out_names),
            lowering_input_output_aliases=(),
            sim_require_finite=True,
            sim_require_nnan=True,
            nc=nc,
        )
        return tuple(outs)

    devices = jax.devices()[:C]
    mesh = Mesh(np.array(devices), ("core",))
    spec = PartitionSpec("core")
    sharded = jax.jit(
        shard_map(
            _body,
            mesh=mesh,
            in_specs=(spec,) * (n_params + n_outs * reps),
            out_specs=(spec,) * n_outs,
            check_rep=False,
        ),
        donate_argnums=donate,
        keep_unused=True,
    )
    shard = NamedSharding(mesh, spec)
    concat_in = [
        jax.device_put(
            np.concatenate([np.asarray(m[nm]) for m in in_maps], axis=0), shard
        )
        for nm in in_names
    ]

    def launch():
        zeros = [
            jax.device_put(np.zeros((C * s[0], *s[1:]), d), shard)
            for (s, d) in zero_shapes
        ]
        return sharded(*concat_in, *zeros)

    def fetch(outs):
        jax.block_until_ready(outs)
        return {
            nm: np.asarray(outs[i]).reshape(C, *out_avals[i].shape)[0]
            for i, nm in enumerate(out_names)
        }

    def run_once():
        return fetch(launch())

    run_once.launch = launch
    run_once.fetch = fetch
    return run_once


def bench(inputs, npc=12500, nreal=100000, iters=6):
    import time

    g = Geo(npc, nreal)
    t0 = time.time()
    in_maps, struct = _preprocess(g, **inputs)
    t1 = time.time()
    nc = _get_prog(g, struct)
    t2 = time.time()
    run_1 = _make_sharded_exec(nc, in_maps)
    out = run_1()  # warmup: compiles + loads NEFF
    t3 = time.time()
    t1s = []
    for _ in range(iters):
        ta = time.time()
        out = run_1()
        t1s.append(time.time() - ta)
    K = 48
    ta = time.time()
    pend = [run_1.launch() for _ in range(K)]
    import jax as _jax
    _jax.block_until_ready(pend)
    tK = time.time() - ta
    per = (tK - min(t1s)) / (K - 1)
    print(
        f"preprocess {t1-t0:.1f}s  build {t2-t1:.1f}s  warmup {t3-t2:.1f}s\n"
        f"  1-shot ms: {[round(t*1e3,2) for t in t1s]}\n"
        f"  {K} pipelined: total {tK*1e3:.1f} ms -> marginal {per*1e3:.3f} ms"
    )
    return np.float32(out["loss"][0, 0]), per
